# revision 30
# baseline (speedup 1.0000x reference)
"""Distributed Trainium2 Bass kernel for nn_AppearancePoolFusion (GNN message passing).

Strategy (v2):
- Edges sharded by dst-node range across 8 cores, dst-sorted, padded per
  128-node group to GC chunks (SPMD-identical graphs).
- Per-edge message msg = Hsrc[src] + Hdst[dst] + e @ We (+b folded into Hdst).
  Hsrc AllGathered, rows fetched per edge with dma_gather using
  prepare_only+trigger so Q7 descriptor generation overlaps compute;
  Hdst expanded per edge with one-hot S^T block matmuls; segment-sum by
  dst with one-hot S block matmuls.
- Pool/pad masks folded into S once per regime change (no per-edge
  mask multiplies); S reloaded from DRAM when an earlier regime returns.
- Edge states kept feature-major: produced by per-chunk PE transposes of
  the f16 message + ACT relu; resident in SBUF for adjacent consumers
  with contiguous DRAM mirrors for later ones (no DMA-transposed loads).
- SAGPool: GAT hs[src] via prepared gather; hs[dst] via S^T matmul
  (dst local, no gather); keep[src] gather folded into S; top-k via
  multi-level 128-ary threshold refinement.
"""

import os
import numpy as np

import concourse.bass as bass
import concourse.bacc as bacc
import concourse.tile as tile
import concourse.mybir as mybir
import concourse.bass_isa as bass_isa
from concourse.bass_utils import run_bass_kernel_spmd
from concourse.masks import make_identity

F32 = mybir.dt.float32
F16 = mybir.dt.float16
I16 = mybir.dt.int16
AF = mybir.ActivationFunctionType
OP = mybir.AluOpType

N = 8192
E = 131072
NCORES = 8
NLOC = N // NCORES      # 1024
NGRP = NLOC // 128      # 8
NF = 128
LRELU = 0.2

TK_LO = -512.0
TK_RANGE = 1024.0
TK_LEVELS = 5

# (name, x_parts, e_parts, wname, out, regime, edge_out, want_deg)
LAYERS = [
    ("conv1",  ["x0"],           ["e0"],         "w1",  128, 0, True,  False),
    ("conv2",  ["x1"],           ["e1"],         "w2",  128, 0, True,  False),
    ("conv3",  ["x2", "x1"],     ["e2", "e1"],   "w3",  128, 0, True,  False),
    ("conv3p", ["x3"],           ["e3"],         "w3p", 256, 0, True,  False),
    ("convs1", ["xp"],           ["es1"],        "ws1", 256, 1, True,  True),
    ("convss", ["xpp"],          ["es2"],        "wss", 256, 2, False, True),
    ("convss2", ["xss1"],        ["ess1"],       "wss", 256, 2, False, False),
    ("convs2", ["xs22"],         ["es2"],        "ws2", 128, 1, False, False),
    ("conv4",  ["x32", "x2"],    ["e3", "e2"],   "w4",  128, 0, False, False),
    ("conv5",  ["x4", "x32"],    ["e4", "e3"],   "w4",  128, 0, False, False),
]
LAYER_XOUT = {"conv1": "x1", "conv2": "x2", "conv3": "x3", "conv3p": "xs1",
              "convs1": "xs2", "convss": "xss1", "convss2": "xss2",
              "convs2": "xs3", "conv4": "x4", "conv5": "x5"}
LAYER_EOUT = {"conv1": "e1", "conv2": "e2", "conv3": "e3", "conv3p": "es1",
              "convs1": "es2", "convss": "ess1", "conv4": "e4"}
BNAME = {"w1": "b1", "w2": "b2", "w3": "b3", "w3p": "b3p", "ws1": "bs1",
         "wss": "bss", "ws2": "bs2", "w4": "b4"}
# producer layer -> name of fused aggregate A = S'^T @ relu(msg) [nl, Fe]
FUSESPEC = {"conv3": "A5b", "convs1": "As2", "convss": "Ass2", "conv4": "A5a"}
# term layers: aggregation-only, no per-edge work, no gather
# aparts: (stash name, weh kt offset, KT of stash)
TERMSPEC = {
    "convss2": dict(aparts=[("Ass2", 0, 2)]),
    "convs2": dict(aparts=[("As2", 0, 2)]),
    "conv5": dict(aparts=[("A5a", 0, 1), ("A5b", 1, 1)]),
}

# e-state placement: where the feature-major state lives.
#   sbuf  : resident SBUF tile only
#   sbufm : resident SBUF tile + DRAM mirror (for later stream consumers)
#   dram  : DRAM mirror only (staged per group on produce, streamed on use)
EPLACE = {"e1": "sbufm", "e2": "sbufm", "e3": "sbufm",
          "es1": "dram", "es2": "dram", "ess1": "dram", "e4": "sbuf"}
# which SBUF slot each resident e-state uses (two rotating 34.8KB slots)
ESLOT = {"e1": "ea", "e2": "eb", "e3": "ea", "e4": "eb"}
# per (layer, epart): consume from sbuf tile or stream from DRAM mirror
ECONSUME = {
    ("conv2", "e1"): "sbuf",
    ("conv3", "e2"): "sbuf", ("conv3", "e1"): "stream",
    ("conv3p", "e3"): "sbuf",
    ("convs1", "es1"): "stream",
    ("convss", "es2"): "stream",
    ("convss2", "ess1"): "stream",
    ("convs2", "es2"): "stream",
    ("conv4", "e3"): "stream", ("conv4", "e2"): "stream",
    ("conv5", "e4"): "sbuf", ("conv5", "e3"): "stream",
}


def _wrap16(idx):
    n = len(idx)
    assert n % 16 == 0
    w = idx.reshape(n // 16, 16).T.astype(np.int16)
    return np.tile(w, (8, 1))


def preprocess(inputs):
    src = np.asarray(inputs["edge_index"])[0].astype(np.int64)
    dst = np.asarray(inputs["edge_index"])[1].astype(np.int64)
    node_feat = np.asarray(inputs["node_feat"], np.float32)
    edge_feat = np.asarray(inputs["edge_feat"], np.float32)

    maxg = 0
    per_core = []
    for c in range(NCORES):
        lo = c * NLOC
        sel = (dst >= lo) & (dst < lo + NLOC)
        s, d = src[sel], dst[sel] - lo
        order = np.argsort(d, kind="stable")
        s, d = s[order], d[order]
        per_core.append((s, d, edge_feat[sel][order]))
        for g in range(NGRP):
            maxg = max(maxg, int(((d >= g * 128) & (d < (g + 1) * 128)).sum()))
    GC = (maxg + 127) // 128
    EC = NGRP * GC * 128
    NCHUNK = NGRP * GC
    meta = dict(GC=GC, EC=EC, NCHUNK=NCHUNK)

    def wtile(Wb, dt=np.float32):
        k, out = Wb.shape
        KT = (k + 127) // 128
        arr = np.zeros((128, KT, out), np.float32)
        for kt in range(KT):
            blk = Wb[kt * 128:(kt + 1) * 128]
            arr[:blk.shape[0], kt, :] = blk
        return arr.astype(dt)

    weights = {}
    for nm, in_x, in_e in [("w1", 64, 64), ("w2", 128, 128), ("w3", 256, 256),
                           ("w3p", 128, 128), ("ws1", 256, 256), ("wss", 256, 256),
                           ("ws2", 256, 256), ("w4", 256, 256)]:
        W = np.asarray(inputs[nm], np.float32)
        weights[nm] = dict(src=wtile(W[:in_x]), dst=wtile(W[in_x:2 * in_x]),
                           e=wtile(W[2 * in_x:]), in_x=in_x, in_e=in_e)

    in_maps = []
    gsz = GC * 128
    for c in range(NCORES):
        s, d, ef = per_core[c]
        lo = c * NLOC
        slot_src = np.zeros(EC, np.int64)
        slot_dstl = np.zeros(EC, np.int64)
        padmask = np.zeros(EC, np.float32)
        e0 = np.zeros((EC, edge_feat.shape[1]), np.float32)
        pos = 0
        for g in range(NGRP):
            gsel = (d >= g * 128) & (d < (g + 1) * 128)
            n = int(gsel.sum())
            base = g * gsz
            slot_src[base:base + n] = s[gsel]
            slot_dstl[base:base + n] = d[gsel]
            slot_dstl[base + n:base + gsz] = g * 128
            padmask[base:base + n] = 1.0
            e0[base:base + n] = ef[pos:pos + n]
            pos += n

        S = np.zeros((128, NCHUNK, 128), np.float16)
        ST = np.zeros((128, NCHUNK, 128), np.float16)
        ch = np.arange(EC) // 128
        pp = np.arange(EC) % 128
        nl = (slot_dstl - (ch // GC) * 128).astype(np.int64)
        valid = padmask > 0
        S[pp[valid], ch[valid], nl[valid]] = 1.0
        ST[nl[valid], ch[valid], pp[valid]] = 1.0

        deg0 = np.bincount(slot_dstl[valid], minlength=NLOC).astype(np.float32)
        recip0 = (1.0 / np.maximum(deg0, 1.0)).astype(np.float32)

        CT = np.zeros((128, NGRP, 64, 128), np.float32)
        np.add.at(CT, (s % 128, d // 128, s // 128, d % 128), 1.0)
        CT = CT.astype(np.float16)

        src_full = _wrap16(slot_src.astype(np.int16))

        pmask = np.zeros((128, NCHUNK), np.float32)
        pmask[pp[valid], ch[valid]] = 1.0

        xl = node_feat[lo:lo + NLOC]
        x0_fm = np.zeros((128, NGRP * 128), np.float32)
        for g in range(NGRP):
            x0_fm[:64, g * 128:(g + 1) * 128] = xl[g * 128:(g + 1) * 128].T

        steps = np.zeros((128, TK_LEVELS), np.float32)
        for l in range(TK_LEVELS):
            steps[:, l] = (np.arange(128) + 1) * (TK_RANGE / (128.0 ** (l + 1)))
        stepsr = np.zeros((1, TK_LEVELS * 128), np.float32)
        for l in range(TK_LEVELS):
            stepsr[0, l * 128:(l + 1) * 128] = \
                (np.arange(128) + 1) * (TK_RANGE / (128.0 ** (l + 1)))

        e0fm_hi = np.zeros((128, EC), np.float16)
        e0fm_hi[:64] = e0.astype(np.float16).T

        m = dict(
            S=S, ST=ST,
            SRCF=src_full,
            PMASK=pmask,
            RECIP0=recip0.reshape(NGRP, 128).T.copy(),
            DEG0=deg0.reshape(NGRP, 128).T.copy(),
            CT=CT,
            X0FM=x0_fm,
            E0FMH=e0fm_hi,
            STEPS=steps,
            STEPSR=stepsr,
        )
        for nm, wd in weights.items():
            m[f"{nm}_src16"] = wd["src"].astype(np.float16).reshape(128, -1)
            m[f"{nm}_dst16"] = wd["dst"].astype(np.float16).reshape(128, -1)
            m[f"{nm}_eh"] = wd["e"].astype(np.float16).reshape(128, -1)
        for nm in ["b1", "b2", "b3", "b3p", "bs1", "bss", "bs2", "b4", "bl"]:
            b = np.asarray(inputs[nm], np.float32)
            m[nm] = np.tile(b.reshape(1, -1), (128, 1))
        for nm in ["wg1", "wg2"]:
            m[nm] = wtile(np.asarray(inputs[nm], np.float32)).reshape(128, -1)
        for i, nm in enumerate(["ag1", "ag2"]):
            a = np.asarray(inputs[nm], np.float32)
            bgv = float(np.asarray(inputs["bg1" if i == 0 else "bg2"], np.float32)[0])
            m[nm] = np.tile(np.array([[a[0], a[1], a[0] + a[1], bgv]], np.float32), (128, 1))
        m["wl"] = np.asarray(inputs["wl"], np.float32)
        in_maps.append(m)

    return meta, in_maps, weights


def build(meta, weights):
    GC, EC, NCHUNK = meta["GC"], meta["EC"], meta["NCHUNK"]
    gsz = GC * 128
    HB = NGRP // 2          # groups per gather call (2 calls/round)
    hsz = HB * gsz          # idxs per gather call

    nc = bacc.Bacc(None, target_bir_lowering=False)

    P = {}

    def param(name, shape, dtype=F32):
        P[name] = nc.declare_dram_parameter(name, list(shape), dtype, isOutput=False)
        return P[name]

    param("S", [128, NCHUNK, 128], F16)
    param("ST", [128, NCHUNK, 128], F16)
    param("SRCF", [128, EC // 16], I16)
    param("PMASK", [128, NCHUNK], F32)
    param("RECIP0", [128, NGRP], F32)
    param("DEG0", [128, NGRP], F32)
    param("CT", [128, NGRP, 64, 128], F16)
    param("X0FM", [128, NGRP * 128], F32)
    param("E0FMH", [128, EC], F16)
    param("STEPS", [128, TK_LEVELS], F32)
    param("STEPSR", [1, TK_LEVELS * 128], F32)
    for nm, wd in weights.items():
        KTx = wd["src"].shape[1]
        KTe = wd["e"].shape[1]
        out = wd["src"].shape[2]
        param(f"{nm}_src16", [128, KTx * out], F16)
        param(f"{nm}_dst16", [128, KTx * out], F16)
        param(f"{nm}_eh", [128, KTe * out], F16)
    for nm, dd in [("b1", 128), ("b2", 128), ("b3", 128), ("b3p", 256),
                   ("bs1", 256), ("bss", 256), ("bs2", 128), ("b4", 128), ("bl", 4)]:
        param(nm, [128, dd], F32)
    param("wg1", [128, 2], F32)
    param("wg2", [128, 2], F32)
    param("ag1", [128, 4], F32)
    param("ag2", [128, 4], F32)
    param("wl", [128, 4], F32)

    OUT = nc.declare_dram_parameter("out", [128, NGRP, 4], F32, isOutput=True)

    LB = {nm: dict(wd) for nm, wd in weights.items()}

    with tile.TileContext(nc) as tc:
        psum = tc.alloc_tile_pool(name="ps", bufs=1, space="PSUM")
        persist = tc.alloc_tile_pool(name="persist", bufs=1)
        epool = tc.alloc_tile_pool(name="epool", bufs=1)
        dram = tc.alloc_tile_pool(name="dram", bufs=1, space="DRAM")

        gat_sem = nc.alloc_semaphore("gat_dma")

        def pload(name, shape, dtype=F32):
            t = persist.tile(list(shape), dtype, tag=name, name=name)
            nc.sync.dma_start(t[:], P[name][:])
            return t

        S_sb = pload("S", [128, NCHUNK, 128], F16)
        SRCF_sb = pload("SRCF", [128, EC // 16], I16)
        PMASK_sb = pload("PMASK", [128, NCHUNK], F32)
        RECIP0_sb = pload("RECIP0", [128, NGRP], F32)
        DEG0_sb = pload("DEG0", [128, NGRP], F32)
        STEPS_sb = pload("STEPS", [128, TK_LEVELS], F32)
        STEPSR_sb = pload("STEPSR", [1, TK_LEVELS * 128], F32)
        ag1_sb = pload("ag1", [128, 4], F32)
        ag2_sb = pload("ag2", [128, 4], F32)

        dum16 = persist.tile([1, 64], F16, tag="dum16", name="dum16")
        dum32 = persist.tile([1, 64], F32, tag="dum32", name="dum32")
        dumb = persist.tile([128, 64], F32, tag="dumb", name="dumb")
        ident = persist.tile([128, 128], F32, tag="ident", name="ident")
        make_identity(nc, ident[:])
        ident16 = persist.tile([128, 128], F16, tag="ident16", name="ident16")
        nc.vector.tensor_copy(ident16[:], ident[:])

        # node state: name -> dict(dram=[128, NGRP*F] f32 DRAM tile, F)
        state = {"x0": dict(dram=None, F=128)}
        # e-state: name -> dict(F, KT, kind, sb=tile|None, mir=dram|None)
        estate = {"e0": dict(F=128, KT=1, kind="host")}

        recips = {0: RECIP0_sb}
        degs = {0: DEG0_sb}
        astash = {}
        poolkeep = {}
        keeps = {0: None}
        pmasks = {0: PMASK_sb}   # [128, NCHUNK] masks for pool `we` weighting

        def new_estate(name, F):
            KT = F // 128
            kind = EPLACE[name]
            d = dict(F=F, KT=KT, kind=kind, sb=None, mir=None)
            if kind in ("sbuf", "sbufm"):
                d["sb"] = epool.tile([128, KT, NCHUNK, 128], F16,
                                     tag=ESLOT[name], name=f"esb_{name}")
            if kind in ("sbufm", "dram"):
                d["mir"] = dram.tile([128, KT, NCHUNK, 128], F16,
                                     tag=f"mir_{name}", name=f"mir_{name}")
            estate[name] = d
            return d

        def new_x(name, F):
            t = dram.tile([128, NGRP * F], F32, tag=f"x_{name}", name=f"x_{name}")
            state[name] = dict(dram=t, F=F)
            return t

        def load_nm(pool, xname, tag):
            st = state[xname]
            t = pool.tile([128, NGRP, st["F"]], F32, tag=tag, name=tag)
            nc.sync.dma_start(t[:], st["dram"][:].rearrange(
                "p (g f) -> p g f", g=NGRP, f=st["F"]))
            return t

        def load_fm(pool, xname, dtype, tag):
            """DRAM x_nm -> feature-major [128, KT, NGRP, 128] via PE transpose."""
            st = state[xname]
            F = st["F"]
            if xname == "x0":
                fm = pool.tile([128, 1, NGRP, 128], dtype, tag=tag, name=tag)
                if dtype == F32:
                    nc.sync.dma_start(fm[:], P["X0FM"][:])
                else:
                    tmp = pool.tile([128, 1, NGRP, 128], F32, tag=tag + "_t", name=tag + "_t")
                    nc.sync.dma_start(tmp[:], P["X0FM"][:])
                    nc.vector.tensor_copy(fm[:], tmp[:])
                return fm
            KT = F // 128
            xnm = load_nm(pool, xname, tag + "_nm")
            fm = pool.tile([128, KT, NGRP, 128], dtype, tag=tag, name=tag)
            for g in range(NGRP):
                for kt in range(KT):
                    tg = "ps_trA" if (g * KT + kt) % 2 == 0 else "ps_trB"
                    pt = psum.tile([128, 128], F32, tag=tg, name=tg)
                    nc.tensor.transpose(pt[:], xnm[:, g, kt * 128:(kt + 1) * 128], ident[:])
                    nc.vector.tensor_copy(fm[:, kt, g, :], pt[:])
            return fm

        def reload_S():
            nc.sync.dma_start(S_sb[:], P["S"][:])

        # ---------------- conv layer ----------------
        def conv_layer(li, name, x_parts, e_parts, wname, out, regime, edge_out,
                       want_deg, keepreg=None):
            wd = LB[wname]
            KTx = wd["src"].shape[1]
            KTe = wd["e"].shape[1]
            outP = out + 1 if want_deg else out
            wide = out + 128 if keepreg else out

            lp = tc.alloc_tile_pool(name=f"L{li}", bufs=1)
            lps = tc.alloc_tile_pool(name=f"L{li}d", bufs=2)
            lps1 = tc.alloc_tile_pool(name=f"L{li}s", bufs=1)

            w16s = lp.tile([128, KTx, out], F16, tag="w16s", name="w16s")
            nc.sync.dma_start(w16s[:], P[f"{wname}_src16"][:])
            w16d = lp.tile([128, KTx, out], F16, tag="w16d", name="w16d")
            nc.sync.dma_start(w16d[:], P[f"{wname}_dst16"][:])
            weh = lp.tile([128, KTe, out], F16, tag="weh", name="weh")
            nc.sync.dma_start(weh[:], P[f"{wname}_eh"][:])
            brep = lp.tile([128, out], F32, tag="brep", name="brep")
            nc.sync.dma_start(brep[:], P[BNAME[wname]][:])

            # --- node-side H tables (per-group x loads) ---
            x0fm16 = None
            if "x0" in x_parts:
                x0t = lp.tile([128, NGRP * 128], F32, tag="x0t", name="x0t")
                nc.sync.dma_start(x0t[:], P["X0FM"][:])
                x0fm16 = lp.tile([128, NGRP * 128], F16, tag="x0f16", name="x0f16")
                nc.vector.tensor_copy(x0fm16[:], x0t[:])
            hsrc_sb = lp.tile([128, NGRP, out], F16, tag="hsrc_sb", name="hsrc_sb")
            hdst_sb = lp.tile([128, NGRP, out], F16, tag="hdst_sb", name="hdst_sb")
            for g in range(NGRP):
                fmg = []
                for xi, xp_ in enumerate(x_parts):
                    if xp_ == "x0":
                        fmg.append(("x0",))
                        continue
                    F = state[xp_]["F"]
                    KTp = F // 128
                    xg = lps.tile([128, 256], F32, tag=f"xg{xi}", name=f"xg{xi}")
                    nc.sync.dma_start(xg[:, :F],
                                      state[xp_]["dram"][:, g * F:(g + 1) * F])
                    fg = lps.tile([128, 2, 128], F16, tag=f"fg{xi}", name=f"fg{xi}")
                    for kt in range(KTp):
                        tg = "ps_trA" if kt % 2 == 0 else "ps_trB"
                        pt = psum.tile([128, 128], F32, tag=tg, name=tg)
                        nc.tensor.transpose(pt[:], xg[:, kt * 128:(kt + 1) * 128],
                                            ident[:])
                        nc.vector.tensor_copy(fg[:, kt, :], pt[:])
                    fmg.append(("t", fg))

                def fm_ap(xi, kt):
                    if fmg[xi][0] == "x0":
                        return x0fm16[:, g * 128:(g + 1) * 128]
                    return fmg[xi][1][:, kt, :]

                ps_s = psum.tile([128, out], F32, tag="ps_node", name="ps_node")
                ps_d = psum.tile([128, out], F32, tag="ps_node2", name="ps_node2")
                kt_glob = 0
                for xi, xp_ in enumerate(x_parts):
                    KTp = state[xp_]["F"] // 128
                    for kt in range(KTp):
                        last = (kt_glob == KTx - 1)
                        nc.tensor.matmul(ps_s[:], fm_ap(xi, kt), w16s[:, kt_glob, :],
                                         start=(kt_glob == 0), stop=last)
                        nc.tensor.matmul(ps_d[:], fm_ap(xi, kt), w16d[:, kt_glob, :],
                                         start=(kt_glob == 0), stop=last)
                        kt_glob += 1
                nc.vector.tensor_copy(hsrc_sb[:, g, :], ps_s[:])
                t1 = lps1.tile([128, out], F32, tag="hdtmp", name="hdtmp")
                nc.vector.tensor_tensor(t1[:], ps_d[:], brep[:], op=OP.add)
                nc.vector.tensor_copy(hdst_sb[:, g, :], t1[:])

            # --- AllGather Hsrc (optionally with pool-keep column) ---
            ag_in = dram.tile([NLOC, wide], F16, tag="ag_in", name="ag_in")
            ag_out = dram.tile([N, wide], F16, tag="ag_out", name="ag_out",
                               addr_space="Shared")
            agv_in = ag_in[:].rearrange("(g p) f -> p g f", p=128, g=NGRP)
            nc.sync.dma_start(agv_in[:, :, :out], hsrc_sb[:])
            if keepreg:
                nc.sync.dma_start(agv_in[:, :, out:out + 1],
                                  poolkeep[keepreg][:].unsqueeze(2))
            nc.gpsimd.collective_compute(
                "AllGather", OP.bypass, replica_groups=[list(range(NCORES))],
                ins=[ag_in[:].opt()], outs=[ag_out[:].opt()])

            # --- gathers: prepare early, trigger after AllGather ---
            hgs = []
            for h in range(NGRP):
                hg = lp.tile([128, GC, wide], F16, tag=f"hg{h % 2}",
                             name=f"hg{h % 2}")
                nc.gpsimd.dma_gather(
                    out_ap=hg[:], in_ap=ag_out[:],
                    idxs_ap=SRCF_sb[:, h * (gsz // 16):(h + 1) * (gsz // 16)],
                    num_idxs=gsz, num_idxs_reg=gsz, elem_size=wide,
                    single_packet=False)
                hgs.append(hg)

            if edge_out:
                eo = new_estate(LAYER_EOUT[name], out)
            aggsb = lp.tile([128, NGRP, outP], F16, tag="aggsb", name="aggsb")
            fuse = FUSESPEC.get(name)
            if fuse:
                stash = persist.tile([128, NGRP, out], F16, tag=f"ast_{fuse}",
                                     name=f"ast_{fuse}")
                astash[fuse] = stash

            # --- edge phase ---
            for g in range(NGRP):
                # edge-feature (feature-major) inputs for this group
                efm_slices = []   # list of (tile, index-fn) per e_part kt
                for ei, ep in enumerate(e_parts):
                    ed = estate[ep]
                    if ed["kind"] == "host":
                        t = lps.tile([128, gsz], F16, tag=f"efm{ei}h", name=f"efm{ei}h")
                        nc.sync.dma_start(t[:], P["E0FMH"][:, g * gsz:(g + 1) * gsz])
                        efm_slices.append(("host", t))
                    elif ECONSUME[(name, ep)] == "sbuf":
                        efm_slices.append(("sbuf", ed["sb"], ed["KT"]))
                    else:
                        t = lps.tile([128, ed["KT"], GC, 128], F16,
                                     tag=f"efm{ei}s", name=f"efm{ei}s")
                        nc.sync.dma_start(
                            t[:], ed["mir"][:, :, g * GC:(g + 1) * GC, :])
                        efm_slices.append(("stream", t, ed["KT"]))

                def efm_ap(t_in_g, kt_glob):
                    k = kt_glob
                    for es in efm_slices:
                        if es[0] == "host":
                            if k == 0:
                                return es[1][:, t_in_g * 128:(t_in_g + 1) * 128]
                            k -= 1
                        elif es[0] == "sbuf":
                            if k < es[2]:
                                return es[1][:, k, g * GC + t_in_g, :]
                            k -= es[2]
                        else:
                            if k < es[2]:
                                return es[1][:, k, t_in_g, :]
                            k -= es[2]
                    raise AssertionError

                stg = lps1.tile([128, GC, 128], F16, tag="stg", name="stg")
                nc.sync.dma_start(stg[:], P["ST"][:, g * GC:(g + 1) * GC, :])

                mpool = lps1 if (fuse and edge_out and out == 256) else lps
                mgo = mpool.tile([128, GC, outP], F16, tag="mgo", name="mgo")
                if want_deg:
                    nc.vector.memset(mgo[:, :, out:outP], 1.0)
                if fuse:
                    ego = lps1.tile([128, GC, out], F16, tag="ego", name="ego")

                if edge_out and eo["kind"] == "dram":
                    fmstage = lps1.tile([128, eo["KT"], GC, 128], F16,
                                       tag="fmstage", name="fmstage")

                hgt = hgs[g]
                if keepreg:
                    if g == 0:
                        sm16 = persist.tile([128, NCHUNK], F16,
                                            tag=f"smask16_{keepreg}",
                                            name=f"smask16_{keepreg}")
                        pmasks[keepreg] = sm16
                    nc.vector.tensor_tensor(sm16[:, g * GC:(g + 1) * GC],
                                            hgt[:, :, out],
                                            PMASK_sb[:, g * GC:(g + 1) * GC],
                                            op=OP.mult)
                    nc.vector.tensor_tensor(
                        S_sb[:, g * GC:(g + 1) * GC, :],
                        S_sb[:, g * GC:(g + 1) * GC, :],
                        sm16[:, g * GC:(g + 1) * GC].unsqueeze(2).to_broadcast(
                            [128, GC, 128]),
                        op=OP.mult)
                ps_agg = psum.tile([128, outP], F32, tag="ps_agg", name="ps_agg")
                if fuse:
                    a_ps = psum.tile([128, out], F32, tag="ps_node", name="ps_node")
                for t in range(GC):
                    c = g * GC + t
                    ptag = "ps_msgA" if t % 2 == 0 else "ps_msgB"
                    pm = psum.tile([128, out], F32, tag=ptag, name=ptag)
                    for kt in range(KTe):
                        nc.tensor.matmul(pm[:], efm_ap(t, kt),
                                         weh[:, kt, :], start=(kt == 0), stop=False)
                    nc.tensor.matmul(pm[:], stg[:, t, :], hdst_sb[:, g, :],
                                     start=False, stop=True)
                    nc.vector.tensor_tensor(mgo[:, t, :out], pm[:],
                                            hgt[:, t, :out], op=OP.add)
                    if edge_out:
                        for kt in range(out // 128):
                            tg = "ps_trA" if (t * 2 + kt) % 2 == 0 else "ps_trB"
                            pt = psum.tile([128, 128], F16, tag=tg, name=tg)
                            nc.tensor.transpose(
                                pt[:], mgo[:, t, kt * 128:(kt + 1) * 128], ident16[:])
                            dst_ap = (fmstage[:, kt, t, :]
                                      if eo["kind"] == "dram"
                                      else eo["sb"][:, kt, c, :])
                            nc.scalar.activation(dst_ap, pt[:], AF.Relu)
                    nc.tensor.matmul(ps_agg[:], S_sb[:, c, :], mgo[:, t, :],
                                     start=(t == 0), stop=(t == GC - 1))
                    if fuse:
                        nc.scalar.activation(ego[:, t, :], mgo[:, t, :out], AF.Relu)
                        nc.tensor.matmul(a_ps[:], S_sb[:, c, :], ego[:, t, :],
                                         start=(t == 0), stop=(t == GC - 1))
                nc.vector.tensor_copy(aggsb[:, g, :], ps_agg[:])
                if fuse:
                    nc.vector.tensor_copy(stash[:, g, :], a_ps[:])
                if edge_out:
                    if eo["kind"] == "dram":
                        nc.sync.dma_start(
                            eo["mir"][:, :, g * GC:(g + 1) * GC, :], fmstage[:])
                    elif eo["kind"] == "sbufm":
                        nc.sync.dma_start(
                            eo["mir"][:, :, g * GC:(g + 1) * GC, :],
                            eo["sb"][:, :, g * GC:(g + 1) * GC, :])

            # --- node update ---
            xout = LAYER_XOUT[name]
            if want_deg:
                dsum = persist.tile([128, NGRP], F32, tag=f"degsum{regime}",
                                    name=f"degsum{regime}")
                nc.vector.tensor_copy(dsum[:], aggsb[:, :, out])
                kp = keeps[regime]
                ddt = lp.tile([128, NGRP], F32, tag="ddt", name="ddt")
                nc.vector.tensor_tensor(ddt[:], dsum[:], kp[:], op=OP.mult)
                nc.vector.tensor_scalar(ddt[:], ddt[:], 1.0, None, op0=OP.max)
                rec = persist.tile([128, NGRP], F32, tag=f"recip{regime}",
                                   name=f"recip{regime}")
                nc.vector.reciprocal(rec[:], ddt[:])
                recips[regime] = rec
                degs[regime] = dsum
            rec = recips[regime]
            kp = keeps[regime]
            fac = lp.tile([128, NGRP], F32, tag="fac", name="fac")
            if kp is not None:
                nc.vector.tensor_tensor(fac[:], rec[:], kp[:], op=OP.mult)
            else:
                nc.vector.tensor_copy(fac[:], rec[:])

            xtmp = lp.tile([128, NGRP, out], F32, tag="xtmp", name="xtmp")
            nc.vector.tensor_tensor(xtmp[:], aggsb[:, :, :out],
                                    fac[:].unsqueeze(2).to_broadcast([128, NGRP, out]),
                                    op=OP.mult)
            nc.scalar.activation(xtmp[:], xtmp[:], AF.Relu)
            xd = new_x(xout, out)
            nc.sync.dma_start(xd[:].rearrange("p (g f) -> p g f", g=NGRP, f=out), xtmp[:])

            lps1.release()
            lps.release()
            lp.release()

        # ---------------- term layer (aggregation only, no gather) ----------
        def term_layer(li, name, x_parts, wname, out, regime):
            wd = LB[wname]
            KTx = wd["src"].shape[1]
            KTe = wd["e"].shape[1]
            aparts = TERMSPEC[name]["aparts"]

            lp = tc.alloc_tile_pool(name=f"T{li}", bufs=1)
            lps = tc.alloc_tile_pool(name=f"T{li}d", bufs=2)
            lps1 = tc.alloc_tile_pool(name=f"T{li}s", bufs=1)

            w16s = lp.tile([128, KTx, out], F16, tag="w16s", name="w16s")
            nc.sync.dma_start(w16s[:], P[f"{wname}_src16"][:])
            w16d = lp.tile([128, KTx, out], F16, tag="w16d", name="w16d")
            nc.sync.dma_start(w16d[:], P[f"{wname}_dst16"][:])
            weh = lp.tile([128, KTe, out], F16, tag="weh", name="weh")
            nc.sync.dma_start(weh[:], P[f"{wname}_eh"][:])
            brep = lp.tile([128, out], F32, tag="brep", name="brep")
            nc.sync.dma_start(brep[:], P[BNAME[wname]][:])

            hsrc_sb = lp.tile([128, NGRP, out], F16, tag="hsrc_sb", name="hsrc_sb")
            hdst_sb = lp.tile([128, NGRP, out], F16, tag="hdst_sb", name="hdst_sb")
            for g in range(NGRP):
                fmg = []
                for xi, xp_ in enumerate(x_parts):
                    F = state[xp_]["F"]
                    KTp = F // 128
                    xg = lps.tile([128, 256], F32, tag=f"xg{xi}", name=f"xg{xi}")
                    nc.sync.dma_start(xg[:, :F],
                                      state[xp_]["dram"][:, g * F:(g + 1) * F])
                    fg = lps.tile([128, 2, 128], F16, tag=f"fg{xi}", name=f"fg{xi}")
                    for kt in range(KTp):
                        tg = "ps_trA" if kt % 2 == 0 else "ps_trB"
                        pt = psum.tile([128, 128], F32, tag=tg, name=tg)
                        nc.tensor.transpose(pt[:], xg[:, kt * 128:(kt + 1) * 128],
                                            ident[:])
                        nc.vector.tensor_copy(fg[:, kt, :], pt[:])
                    fmg.append(fg)
                ps_s = psum.tile([128, out], F32, tag="ps_node", name="ps_node")
                ps_d = psum.tile([128, out], F32, tag="ps_node2", name="ps_node2")
                kt_glob = 0
                for xi, xp_ in enumerate(x_parts):
                    KTp = state[xp_]["F"] // 128
                    for kt in range(KTp):
                        last = (kt_glob == KTx - 1)
                        nc.tensor.matmul(ps_s[:], fmg[xi][:, kt, :], w16s[:, kt_glob, :],
                                         start=(kt_glob == 0), stop=last)
                        nc.tensor.matmul(ps_d[:], fmg[xi][:, kt, :], w16d[:, kt_glob, :],
                                         start=(kt_glob == 0), stop=last)
                        kt_glob += 1
                nc.vector.tensor_copy(hsrc_sb[:, g, :], ps_s[:])
                t1 = lps1.tile([128, out], F32, tag="hdtmp", name="hdtmp")
                nc.vector.tensor_tensor(t1[:], ps_d[:], brep[:], op=OP.add)
                nc.vector.tensor_copy(hdst_sb[:, g, :], t1[:])

            ag_in = dram.tile([NLOC, out], F16, tag="ag_in", name="ag_in")
            ag_out = dram.tile([N, out], F16, tag="ag_out", name="ag_out",
                               addr_space="Shared")
            nc.sync.dma_start(ag_in[:].rearrange("(g p) f -> p g f", p=128, g=NGRP),
                              hsrc_sb[:])
            nc.gpsimd.collective_compute(
                "AllGather", OP.bypass, replica_groups=[list(range(NCORES))],
                ins=[ag_in[:].opt()], outs=[ag_out[:].opt()])

            tbl = lp.tile([128, 64, out], F16, tag="tbl", name="tbl")
            nc.sync.dma_start(tbl[:],
                              ag_out[:].rearrange("(c p) f -> p c f", c=64, p=128))

            degt = degs[regime]
            aggsb = lp.tile([128, NGRP, out], F32, tag="aggsb", name="aggsb")
            for g in range(NGRP):
                ctg = lps.tile([128, 64, 128], F16, tag="ctg", name="ctg")
                nc.sync.dma_start(ctg[:], P["CT"][:, g, :, :])
                ats = []
                for ai, (anm, woff, KTa) in enumerate(aparts):
                    st_t = astash[anm]
                    at = lps.tile([128, 2, 128], F16, tag=f"at{ai}", name=f"at{ai}")
                    for kt in range(KTa):
                        tg = "ps_trA" if kt % 2 == 0 else "ps_trB"
                        pt = psum.tile([128, 128], F16, tag=tg, name=tg)
                        nc.tensor.transpose(pt[:], st_t[:, g, kt * 128:(kt + 1) * 128],
                                            ident16[:])
                        nc.vector.tensor_copy(at[:, kt, :], pt[:])
                    ats.append((at, woff, KTa))
                ps_a = psum.tile([128, out], F32, tag="ps_agg", name="ps_agg")
                for k in range(64):
                    nc.tensor.matmul(ps_a[:], ctg[:, k, :], tbl[:, k, :],
                                     start=(k == 0), stop=False)
                nmm = sum(KTa for _, _, KTa in aparts)
                j = 0
                for at, woff, KTa in ats:
                    for kt in range(KTa):
                        j += 1
                        nc.tensor.matmul(ps_a[:], at[:, kt, :], weh[:, woff + kt, :],
                                         start=False, stop=(j == nmm))
                t2 = lps1.tile([128, out], F32, tag="t2g", name="t2g")
                nc.vector.tensor_scalar(t2[:], hdst_sb[:, g, :], degt[:, g:g + 1],
                                        None, op0=OP.mult)
                nc.vector.tensor_tensor(aggsb[:, g, :], ps_a[:], t2[:], op=OP.add)

            xout = LAYER_XOUT[name]
            rec = recips[regime]
            kp = keeps[regime]
            fac = lp.tile([128, NGRP], F32, tag="fac", name="fac")
            if kp is not None:
                nc.vector.tensor_tensor(fac[:], rec[:], kp[:], op=OP.mult)
            else:
                nc.vector.tensor_copy(fac[:], rec[:])
            xtmp = lp.tile([128, NGRP, out], F32, tag="xtmp", name="xtmp")
            nc.vector.tensor_tensor(xtmp[:], aggsb[:],
                                    fac[:].unsqueeze(2).to_broadcast([128, NGRP, out]),
                                    op=OP.mult)
            nc.scalar.activation(xtmp[:], xtmp[:], AF.Relu)
            xd = new_x(xout, out)
            nc.sync.dma_start(xd[:].rearrange("p (g f) -> p g f", g=NGRP, f=out), xtmp[:])
            lps1.release()
            lps.release()
            lp.release()

        # ---------------- GAT pool ----------------
        def gat_pool(pi, xname, wgname, agname, k, regime_in, regime_out, xpout):
            pp = tc.alloc_tile_pool(name=f"P{pi}", bufs=1)
            st = state[xname]
            Fdim = st["F"]
            KT = Fdim // 128
            agv = ag1_sb if pi == 0 else ag2_sb

            wg = pp.tile([128, KT], F32, tag="wg", name="wg")
            nc.sync.dma_start(wg[:], P[wgname][:])
            wg16 = pp.tile([128, KT], F16, tag="wg16", name="wg16")
            nc.vector.tensor_copy(wg16[:], wg[:])

            hs = pp.tile([128, NGRP], F32, tag="hs", name="hs")
            for g in range(NGRP):
                xg = pp.tile([128, 256], F32, tag="gxg", name="gxg")
                nc.sync.dma_start(xg[:, :Fdim],
                                  state[xname]["dram"][:, g * Fdim:(g + 1) * Fdim])
                fg = pp.tile([128, 2, 128], F16, tag="gfg", name="gfg")
                for kt in range(KT):
                    tg = "ps_trA" if kt % 2 == 0 else "ps_trB"
                    pt = psum.tile([128, 128], F32, tag=tg, name=tg)
                    nc.tensor.transpose(pt[:], xg[:, kt * 128:(kt + 1) * 128], ident[:])
                    nc.vector.tensor_copy(fg[:, kt, :], pt[:])
                ph = psum.tile([128, 1], F32, tag="ps_small", name="ps_small")
                for kt in range(KT):
                    nc.tensor.matmul(ph[:], fg[:, kt, :], wg16[:, kt:kt + 1],
                                     start=(kt == 0), stop=(kt == KT - 1))
                nc.vector.tensor_copy(hs[:, g:g + 1], ph[:])
            hs16 = pp.tile([128, NGRP], F16, tag="hs16", name="hs16")
            nc.vector.tensor_copy(hs16[:], hs[:])

            hrep = pp.tile([128, NGRP, 64], F32, tag="hrep", name="hrep")
            nc.vector.tensor_copy(hrep[:], hs[:].unsqueeze(2).to_broadcast([128, NGRP, 64]))
            hs_in = dram.tile([NLOC, 64], F32, tag="hs_ag_in", name="hs_ag_in")
            hs_out = dram.tile([N, 64], F32, tag="hs_ag_out", name="hs_ag_out",
                               addr_space="Shared")
            nc.sync.dma_start(hs_in[:].rearrange("(g p) f -> p g f", p=128, g=NGRP), hrep[:])
            nc.gpsimd.collective_compute(
                "AllGather", OP.bypass, replica_groups=[list(range(NCORES))],
                ins=[hs_in[:].opt()], outs=[hs_out[:].opt()])

            # per-edge hs[src] via prepared gather
            gbig = pp.tile([128, NCHUNK, 64], F32, tag="gbig", name="gbig")
            for h in range(2):
                nc.gpsimd.dma_gather(
                    out_ap=gbig[:, h * (NCHUNK // 2):(h + 1) * (NCHUNK // 2), :],
                    in_ap=hs_out[:],
                    idxs_ap=SRCF_sb[:, h * (hsz // 16):(h + 1) * (hsz // 16)],
                    num_idxs=hsz, num_idxs_reg=hsz, elem_size=64,
                    single_packet=False)
            hsrc_e = pp.tile([128, NCHUNK], F32, tag="hsrc_e", name="hsrc_e")
            nc.vector.tensor_copy(hsrc_e[:], gbig[:, :, 0])

            # per-edge hs[dst] via ST matmul (dst local)
            hdst_e = pp.tile([128, NCHUNK], F32, tag="hdst_e", name="hdst_e")
            for g in range(NGRP):
                stg = pp.tile([128, GC, 128], F16, tag="stgp", name="stgp")
                nc.sync.dma_start(stg[:], P["ST"][:, g * GC:(g + 1) * GC, :])
                pg = psum.tile([128, GC], F32, tag="ps_small", name="ps_small")
                for t in range(GC):
                    nc.tensor.matmul(pg[:, t:t + 1], stg[:, t, :], hs16[:, g:g + 1],
                                     start=True, stop=True)
                nc.vector.tensor_copy(hdst_e[:, g * GC:(g + 1) * GC], pg[:])

            u = pp.tile([128, NCHUNK], F32, tag="u", name="u")
            nc.vector.tensor_scalar(hdst_e[:], hdst_e[:], agv[:, 1:2], None, op0=OP.mult)
            nc.vector.scalar_tensor_tensor(u[:], hsrc_e[:], agv[:, 0:1], hdst_e[:],
                                           op0=OP.mult, op1=OP.add)
            lg = pp.tile([128, NCHUNK], F32, tag="lg", name="lg")
            nc.vector.tensor_scalar(lg[:], u[:], LRELU, None, op0=OP.mult)
            nc.vector.tensor_tensor(lg[:], lg[:], u[:], op=OP.max)
            w = pp.tile([128, NCHUNK], F32, tag="w", name="w")
            nc.scalar.activation(w[:], lg[:], AF.Exp)
            we = pp.tile([128, NCHUNK], F32, tag="we", name="we")
            nc.vector.tensor_tensor(we[:], w[:], pmasks[regime_in][:], op=OP.mult)
            wpair = pp.tile([128, NCHUNK, 2], F16, tag="wpair", name="wpair")
            nc.vector.tensor_copy(wpair[:, :, 0], we[:])
            nc.vector.tensor_tensor(wpair[:, :, 1], we[:], hsrc_e[:], op=OP.mult)

            gsums = pp.tile([128, NGRP, 2], F32, tag="gsums", name="gsums")
            for g in range(NGRP):
                pg = psum.tile([128, 2], F32, tag="ps_small", name="ps_small")
                for t in range(GC):
                    c = g * GC + t
                    nc.tensor.matmul(pg[:], S_sb[:, c, :], wpair[:, c, :],
                                     start=(t == 0), stop=(t == GC - 1))
                nc.vector.tensor_copy(gsums[:, g, :], pg[:])

            us = pp.tile([128, NGRP], F32, tag="us", name="us")
            nc.vector.tensor_scalar(us[:], hs[:], agv[:, 2:3], None, op0=OP.mult)
            ls = pp.tile([128, NGRP], F32, tag="ls", name="ls")
            nc.vector.tensor_scalar(ls[:], us[:], LRELU, None, op0=OP.mult)
            nc.vector.tensor_tensor(ls[:], ls[:], us[:], op=OP.max)
            exs = pp.tile([128, NGRP], F32, tag="exs", name="exs")
            nc.scalar.activation(exs[:], ls[:], AF.Exp)

            kp_in = keeps[regime_in]
            den = pp.tile([128, NGRP], F32, tag="den", name="den")
            num = pp.tile([128, NGRP], F32, tag="num", name="num")
            if kp_in is not None:
                nc.vector.tensor_tensor(den[:], gsums[:, :, 0], kp_in[:], op=OP.mult)
                nc.vector.tensor_tensor(num[:], gsums[:, :, 1], kp_in[:], op=OP.mult)
            else:
                nc.vector.tensor_copy(den[:], gsums[:, :, 0])
                nc.vector.tensor_copy(num[:], gsums[:, :, 1])
            nc.vector.tensor_tensor(den[:], den[:], exs[:], op=OP.add)
            t2 = pp.tile([128, NGRP], F32, tag="t2", name="t2")
            nc.vector.tensor_tensor(t2[:], exs[:], hs[:], op=OP.mult)
            nc.vector.tensor_tensor(num[:], num[:], t2[:], op=OP.add)
            rden = pp.tile([128, NGRP], F32, tag="rden", name="rden")
            nc.vector.reciprocal(rden[:], den[:])
            score = pp.tile([128, NGRP], F32, tag="score", name="score")
            nc.vector.tensor_tensor(score[:], num[:], rden[:], op=OP.mult)
            nc.vector.tensor_scalar(score[:], score[:], agv[:, 3:4], None, op0=OP.add)

            sc_in = dram.tile([NLOC, 1], F32, tag="sc_ag_in", name="sc_ag_in")
            sc_out = dram.tile([N, 1], F32, tag="sc_ag_out", name="sc_ag_out",
                               addr_space="Shared")
            nc.sync.dma_start(sc_in[:].rearrange("(g p) f -> p g f", p=128, g=NGRP),
                              score[:].unsqueeze(2))
            nc.gpsimd.collective_compute(
                "AllGather", OP.bypass, replica_groups=[list(range(NCORES))],
                ins=[sc_in[:].opt()], outs=[sc_out[:].opt()])

            s_full = pp.tile([128, 64], F32, tag="s_full", name="s_full")
            nc.sync.dma_start(s_full[:], sc_out[:].rearrange("(p c) f -> p (c f)",
                                                             p=128, c=64))
            # masked local / full scores
            if pi == 1:
                k1f = keeps["full1"]
                m1 = pp.tile([128, 64], F32, tag="m1", name="m1")
                nc.vector.tensor_tensor(m1[:], s_full[:], k1f[:], op=OP.mult)
                m2 = pp.tile([128, 64], F32, tag="m2", name="m2")
                nc.vector.tensor_scalar(m2[:], k1f[:], 1.0, None, op0=OP.subtract)
                nc.vector.tensor_scalar(m2[:], m2[:], 1e30, None, op0=OP.mult)
                nc.vector.tensor_tensor(s_full[:], m1[:], m2[:], op=OP.add)
                sl1 = pp.tile([128, NGRP], F32, tag="sl1", name="sl1")
                nc.vector.tensor_tensor(sl1[:], score[:], kp_in[:], op=OP.mult)
                sl2 = pp.tile([128, NGRP], F32, tag="sl2", name="sl2")
                nc.vector.tensor_scalar(sl2[:], kp_in[:], 1.0, None, op0=OP.subtract)
                nc.vector.tensor_scalar(sl2[:], sl2[:], 1e30, None, op0=OP.mult)
                score_m = pp.tile([128, NGRP], F32, tag="score_m", name="score_m")
                nc.vector.tensor_tensor(score_m[:], sl1[:], sl2[:], op=OP.add)
            else:
                score_m = score

            # threshold refinement: counts via compact [128, 128, 64] compare
            tlo = pp.tile([1, 1], F32, tag="tlo0", name="tlo0")
            nc.vector.memset(tlo[:], TK_LO)
            for lvl in range(TK_LEVELS):
                wl = TK_RANGE / (128.0 ** (lvl + 1))
                T_row = pp.tile([1, 128], F32, tag=f"Tr{lvl}", name=f"Tr{lvl}")
                nc.vector.tensor_scalar(T_row[:],
                                        STEPSR_sb[0:1, lvl * 128:(lvl + 1) * 128],
                                        tlo[:], None, op0=OP.add)
                T_all = pp.tile([128, 128], F32, tag="T_all", name="T_all")
                nc.gpsimd.partition_broadcast(T_all[:], T_row[:])
                cmp3 = pp.tile([128, 128, 64], F16, tag="cmp3", name="cmp3")
                nc.vector.tensor_tensor(
                    cmp3[:], s_full[:].unsqueeze(1).to_broadcast([128, 128, 64]),
                    T_all[:].unsqueeze(2).to_broadcast([128, 128, 64]), op=OP.is_ge)
                cntp = pp.tile([128, 128], F32, tag="cntp", name="cntp")
                nc.vector.tensor_reduce(cntp[:].unsqueeze(2), cmp3[:],
                                        axis=mybir.AxisListType.X, op=OP.add)
                cnta = pp.tile([128, 128], F32, tag="cnta", name="cnta")
                nc.gpsimd.partition_all_reduce(cnta[:], cntp[:], channels=128,
                                               reduce_op=bass_isa.ReduceOp.add)
                cj = pp.tile([128, 128], F32, tag="cj", name="cj")
                nc.vector.tensor_scalar(cj[:], cnta[:], float(k), None, op0=OP.is_ge)
                sj = pp.tile([128, 1], F32, tag=f"sj{lvl}", name=f"sj{lvl}")
                nc.vector.tensor_reduce(sj[:], cj[:], axis=mybir.AxisListType.X,
                                        op=OP.add)
                tlo2 = pp.tile([1, 1], F32, tag=f"tlo{lvl + 1}", name=f"tlo{lvl + 1}")
                nc.vector.scalar_tensor_tensor(tlo2[:], sj[0:1, :], wl, tlo[:],
                                               op0=OP.mult, op1=OP.add)
                tlo = tlo2

            tlo_b = pp.tile([128, 1], F32, tag="tlob_f", name="tlob_f")
            nc.gpsimd.partition_broadcast(tlo_b[:], tlo[:])

            keep_full = persist.tile([128, 64], F32, tag=f"keep{pi}f", name=f"keep{pi}f")
            nc.vector.tensor_scalar(keep_full[:], s_full[:], tlo_b[:], None, op0=OP.is_ge)
            keep_loc = persist.tile([128, NGRP], F32, tag=f"keep{pi}l", name=f"keep{pi}l")
            nc.vector.tensor_scalar(keep_loc[:], score_m[:], tlo_b[:], None, op0=OP.is_ge)
            keeps["full1" if pi == 0 else "full2"] = keep_full
            keeps[regime_out] = keep_loc

            # keep vector for the next conv layer's merged gather column
            kl16 = persist.tile([128, NGRP], F16, tag=f"plk{regime_out}",
                                name=f"plk{regime_out}")
            nc.vector.tensor_copy(kl16[:], keep_loc[:])
            poolkeep[regime_out] = kl16

            # xp = x * (1 + keep*(tanh(score)-1))
            th = pp.tile([128, NGRP], F32, tag="th", name="th")
            nc.scalar.activation(th[:], score[:], AF.Tanh)
            nc.vector.tensor_scalar(th[:], th[:], 1.0, None, op0=OP.subtract)
            nc.vector.tensor_tensor(th[:], th[:], keep_loc[:], op=OP.mult)
            nc.vector.tensor_scalar(th[:], th[:], 1.0, None, op0=OP.add)
            xin = load_nm(pp, xname, "xin")
            nc.vector.tensor_tensor(xin[:], xin[:],
                                    th[:].unsqueeze(2).to_broadcast([128, NGRP, Fdim]),
                                    op=OP.mult)
            xd = new_x(xpout, Fdim)
            nc.sync.dma_start(xd[:].rearrange("p (g f) -> p g f", g=NGRP, f=Fdim), xin[:])
            pp.release()

        # ---------------- residual ----------------
        def residual(dst_name, a_name, b_name, keep, F):
            rp = tc.alloc_tile_pool(name=f"R{dst_name}", bufs=1)
            a = load_nm(rp, a_name, "ra")
            b = load_nm(rp, b_name, "rb")
            t = rp.tile([128, NGRP, F], F32, tag="rt", name="rt")
            nc.vector.tensor_tensor(t[:], b[:],
                                    keep[:].unsqueeze(2).to_broadcast([128, NGRP, F]),
                                    op=OP.mult)
            nc.vector.tensor_tensor(t[:], t[:], a[:], op=OP.add)
            xd = new_x(dst_name, F)
            nc.sync.dma_start(xd[:].rearrange("p (g f) -> p g f", g=NGRP, f=F), t[:])
            rp.release()

        # ---------------- network ----------------
        klayers = int(os.environ.get("KLAYERS", "99"))
        kpool = int(os.environ.get("KPOOL", "99"))
        smask1_16 = None
        last_x = "x0"
        for li, spec in enumerate(LAYERS):
            if li >= klayers:
                break
            name, x_parts, e_parts, wname, out, regime, edge_out, want_deg = spec
            if name == "conv4":
                reload_S()
            if name in TERMSPEC:
                term_layer(li, name, x_parts, wname, out, regime)
            else:
                kr = regime if name in ("convs1", "convss") else None
                conv_layer(li, name, x_parts, e_parts, wname, out, regime,
                           edge_out, want_deg, keepreg=kr)
            last_x = LAYER_XOUT[name]
            if name == "conv3p" and kpool >= 1:
                gat_pool(0, "xs1", "wg1", "ag1", N // 2, 0, 1, "xp")
                # no SBUF-resident e-states are used past this point
                epool.release()
                last_x = "xp"
            elif name == "convs1" and kpool >= 2:
                gat_pool(1, "xs2", "wg2", "ag2", N // 4, 1, 2, "xpp")
                last_x = "xpp"
            elif name == "convss2":
                residual("xs22", "xs2", "xss2", keeps[2], 256)
            elif name == "convs2":
                residual("x32", "x3", "xs3", keeps[1], 128)

        kdbg = os.environ.get("KDBG", "")

        # ---------------- final linear ----------------
        fp = tc.alloc_tile_pool(name="fin", bufs=1)
        wl_sb = fp.tile([128, 4], F32, tag="wl_sb", name="wl_sb")
        nc.sync.dma_start(wl_sb[:], P["wl"][:])
        bl_sb = fp.tile([128, 4], F32, tag="bl_sb", name="bl_sb")
        nc.sync.dma_start(bl_sb[:], P["bl"][:])
        x5fm = load_fm(fp, "x5", F16, "x5fm")
        wl16 = fp.tile([128, 4], F16, tag="wl16", name="wl16")
        nc.vector.tensor_copy(wl16[:], wl_sb[:])
        outsb = fp.tile([128, NGRP, 4], F32, tag="outsb", name="outsb")
        for g in range(NGRP):
            po = psum.tile([128, 4], F32, tag="ps_small", name="ps_small")
            nc.tensor.matmul(po[:], x5fm[:, 0, g, :], wl16[:], start=True, stop=True)
            nc.vector.tensor_tensor(outsb[:, g, :], po[:], bl_sb[:], op=OP.add)
        if kdbg:
            xd = state[kdbg]["dram"]
            Fd = state[kdbg]["F"]
            nc.sync.dma_start(OUT[:], xd[:].rearrange(
                "p (g f) -> p g f", g=NGRP, f=Fd)[:, :, 0:4])
        else:
            nc.sync.dma_start(OUT[:], outsb[:])
        fp.release()

        dram.release()
        persist.release()
        psum.release()

    nc.compile()
    return nc


LAST_RESULT = None


def kernel(**inputs):
    global LAST_RESULT
    meta, in_maps, weights = preprocess(inputs)
    nc = build(meta, weights)
    res = run_bass_kernel_spmd(nc, in_maps, list(range(NCORES)))
    LAST_RESULT = res
    outs = []
    for c in range(NCORES):
        o = res.results[c]["out"]
        full = np.zeros((NLOC, 4), np.float32)
        for g in range(NGRP):
            full[g * 128:(g + 1) * 128] = o[:, g, :]
        outs.append(full)
    return np.concatenate(outs, axis=0).astype(np.float32)


# revision 32
# speedup vs baseline: 1.1238x; 1.1238x over previous
"""Distributed Trainium2 Bass kernel for nn_AppearancePoolFusion (GNN message passing).

Strategy (v2):
- Edges sharded by dst-node range across 8 cores, dst-sorted, padded per
  128-node group to GC chunks (SPMD-identical graphs).
- Per-edge message msg = Hsrc[src] + Hdst[dst] + e @ We (+b folded into Hdst).
  Hsrc AllGathered, rows fetched per edge with dma_gather using
  prepare_only+trigger so Q7 descriptor generation overlaps compute;
  Hdst expanded per edge with one-hot S^T block matmuls; segment-sum by
  dst with one-hot S block matmuls.
- Pool/pad masks folded into S once per regime change (no per-edge
  mask multiplies); S reloaded from DRAM when an earlier regime returns.
- Edge states kept feature-major: produced by per-chunk PE transposes of
  the f16 message + ACT relu; resident in SBUF for adjacent consumers
  with contiguous DRAM mirrors for later ones (no DMA-transposed loads).
- SAGPool: GAT hs[src] via prepared gather; hs[dst] via S^T matmul
  (dst local, no gather); keep[src] gather folded into S; top-k via
  multi-level 128-ary threshold refinement.
"""

import os
import numpy as np

import concourse.bass as bass
import concourse.bacc as bacc
import concourse.tile as tile
import concourse.mybir as mybir
import concourse.bass_isa as bass_isa
from concourse.bass_utils import run_bass_kernel_spmd
from concourse.masks import make_identity

F32 = mybir.dt.float32
F16 = mybir.dt.float16
I16 = mybir.dt.int16
AF = mybir.ActivationFunctionType
OP = mybir.AluOpType

N = 8192
E = 131072
NCORES = 8
NLOC = N // NCORES      # 1024
NGRP = NLOC // 128      # 8
NF = 128
LRELU = 0.2

TK_LO = -512.0
TK_RANGE = 1024.0
TK_LEVELS = 6

# (name, x_parts, e_parts, wname, out, regime, edge_out, want_deg)
LAYERS = [
    ("conv1",  ["x0"],           ["e0"],         "w1",  128, 0, True,  False),
    ("conv2",  ["x1"],           ["e1"],         "w2",  128, 0, True,  False),
    ("conv3",  ["x2", "x1"],     ["e2", "e1"],   "w3",  128, 0, True,  False),
    ("conv3p", ["x3"],           ["e3"],         "w3p", 256, 0, True,  False),
    ("convs1", ["xp"],           ["es1"],        "ws1", 256, 1, True,  True),
    ("convss", ["xpp"],          ["es2"],        "wss", 256, 2, False, True),
    ("convss2", ["xss1"],        ["ess1"],       "wss", 256, 2, False, False),
    ("convs2", ["xs22"],         ["es2"],        "ws2", 128, 1, False, False),
    ("conv4",  ["x32", "x2"],    ["e3", "e2"],   "w4",  128, 0, False, False),
    ("conv5",  ["x4", "x32"],    ["e4", "e3"],   "w4",  128, 0, False, False),
]
LAYER_XOUT = {"conv1": "x1", "conv2": "x2", "conv3": "x3", "conv3p": "xs1",
              "convs1": "xs2", "convss": "xss1", "convss2": "xss2",
              "convs2": "xs3", "conv4": "x4", "conv5": "x5"}
LAYER_EOUT = {"conv1": "e1", "conv2": "e2", "conv3": "e3", "conv3p": "es1",
              "convs1": "es2", "convss": "ess1", "conv4": "e4"}
BNAME = {"w1": "b1", "w2": "b2", "w3": "b3", "w3p": "b3p", "ws1": "bs1",
         "wss": "bss", "ws2": "bs2", "w4": "b4"}
# producer layer -> name of fused aggregate A = S'^T @ relu(msg) [nl, Fe]
FUSESPEC = {"conv3": "A5b", "convs1": "As2", "convss": "Ass2", "conv4": "A5a"}
# term layers: aggregation-only, no per-edge work, no gather
# aparts: (stash name, weh kt offset, KT of stash)
TERMSPEC = {
    "convss2": dict(aparts=[("Ass2", 0, 2)]),
    "convs2": dict(aparts=[("As2", 0, 2)]),
    "conv5": dict(aparts=[("A5a", 0, 1), ("A5b", 1, 1)]),
}

# e-state placement: where the feature-major state lives.
#   sbuf  : resident SBUF tile only
#   sbufm : resident SBUF tile + DRAM mirror (for later stream consumers)
#   dram  : DRAM mirror only (staged per group on produce, streamed on use)
EPLACE = {"e1": "sbufm", "e2": "sbufm", "e3": "sbufm",
          "es1": "dram", "es2": "dram", "ess1": "dram", "e4": "sbuf"}
# which SBUF slot each resident e-state uses (two rotating 34.8KB slots)
ESLOT = {"e1": "ea", "e2": "eb", "e3": "ea", "e4": "eb"}
# per (layer, epart): consume from sbuf tile or stream from DRAM mirror
ECONSUME = {
    ("conv2", "e1"): "sbuf",
    ("conv3", "e2"): "sbuf", ("conv3", "e1"): "stream",
    ("conv3p", "e3"): "sbuf",
    ("convs1", "es1"): "stream",
    ("convss", "es2"): "stream",
    ("convss2", "ess1"): "stream",
    ("convs2", "es2"): "stream",
    ("conv4", "e3"): "stream", ("conv4", "e2"): "stream",
    ("conv5", "e4"): "sbuf", ("conv5", "e3"): "stream",
}


def _wrap16(idx):
    n = len(idx)
    assert n % 16 == 0
    w = idx.reshape(n // 16, 16).T.astype(np.int16)
    return np.tile(w, (8, 1))


def preprocess(inputs):
    src = np.asarray(inputs["edge_index"])[0].astype(np.int64)
    dst = np.asarray(inputs["edge_index"])[1].astype(np.int64)
    node_feat = np.asarray(inputs["node_feat"], np.float32)
    edge_feat = np.asarray(inputs["edge_feat"], np.float32)

    maxg = 0
    per_core = []
    for c in range(NCORES):
        lo = c * NLOC
        sel = (dst >= lo) & (dst < lo + NLOC)
        s, d = src[sel], dst[sel] - lo
        order = np.argsort(d, kind="stable")
        s, d = s[order], d[order]
        per_core.append((s, d, edge_feat[sel][order]))
        for g in range(NGRP):
            maxg = max(maxg, int(((d >= g * 128) & (d < (g + 1) * 128)).sum()))
    GC = (maxg + 127) // 128
    EC = NGRP * GC * 128
    NCHUNK = NGRP * GC
    meta = dict(GC=GC, EC=EC, NCHUNK=NCHUNK)

    def wtile(Wb, dt=np.float32):
        k, out = Wb.shape
        KT = (k + 127) // 128
        arr = np.zeros((128, KT, out), np.float32)
        for kt in range(KT):
            blk = Wb[kt * 128:(kt + 1) * 128]
            arr[:blk.shape[0], kt, :] = blk
        return arr.astype(dt)

    weights = {}
    for nm, in_x, in_e in [("w1", 64, 64), ("w2", 128, 128), ("w3", 256, 256),
                           ("w3p", 128, 128), ("ws1", 256, 256), ("wss", 256, 256),
                           ("ws2", 256, 256), ("w4", 256, 256)]:
        W = np.asarray(inputs[nm], np.float32)
        weights[nm] = dict(src=wtile(W[:in_x]), dst=wtile(W[in_x:2 * in_x]),
                           e=wtile(W[2 * in_x:]), in_x=in_x, in_e=in_e)

    in_maps = []
    gsz = GC * 128
    for c in range(NCORES):
        s, d, ef = per_core[c]
        lo = c * NLOC
        slot_src = np.zeros(EC, np.int64)
        slot_dstl = np.zeros(EC, np.int64)
        padmask = np.zeros(EC, np.float32)
        e0 = np.zeros((EC, edge_feat.shape[1]), np.float32)
        pos = 0
        for g in range(NGRP):
            gsel = (d >= g * 128) & (d < (g + 1) * 128)
            n = int(gsel.sum())
            base = g * gsz
            slot_src[base:base + n] = s[gsel]
            slot_dstl[base:base + n] = d[gsel]
            slot_dstl[base + n:base + gsz] = g * 128
            padmask[base:base + n] = 1.0
            e0[base:base + n] = ef[pos:pos + n]
            pos += n

        S = np.zeros((128, NCHUNK, 128), np.float16)
        ST = np.zeros((128, NCHUNK, 128), np.float16)
        ch = np.arange(EC) // 128
        pp = np.arange(EC) % 128
        nl = (slot_dstl - (ch // GC) * 128).astype(np.int64)
        valid = padmask > 0
        S[pp[valid], ch[valid], nl[valid]] = 1.0
        ST[nl[valid], ch[valid], pp[valid]] = 1.0

        deg0 = np.bincount(slot_dstl[valid], minlength=NLOC).astype(np.float32)
        recip0 = (1.0 / np.maximum(deg0, 1.0)).astype(np.float32)

        CT = np.zeros((128, NGRP, 64, 128), np.float32)
        np.add.at(CT, (s % 128, d // 128, s // 128, d % 128), 1.0)
        CT = CT.astype(np.float16)

        src_full = _wrap16(slot_src.astype(np.int16))

        pmask = np.zeros((128, NCHUNK), np.float32)
        pmask[pp[valid], ch[valid]] = 1.0

        xl = node_feat[lo:lo + NLOC]
        x0_fm = np.zeros((128, NGRP * 128), np.float32)
        for g in range(NGRP):
            x0_fm[:64, g * 128:(g + 1) * 128] = xl[g * 128:(g + 1) * 128].T

        steps = np.zeros((128, TK_LEVELS), np.float32)
        for l in range(TK_LEVELS):
            steps[:, l] = (np.arange(128) + 1) * (TK_RANGE / (128.0 ** (l + 1)))
        stepsr = np.zeros((1, TK_LEVELS * 128), np.float32)
        for l in range(TK_LEVELS):
            stepsr[0, l * 128:(l + 1) * 128] = \
                (np.arange(128) + 1) * (TK_RANGE / (128.0 ** (l + 1)))

        e0fm_hi = np.zeros((128, EC), np.float16)
        e0fm_hi[:64] = e0.astype(np.float16).T

        m = dict(
            S=S, ST=ST,
            SRCF=src_full,
            PMASK=pmask,
            RECIP0=recip0.reshape(NGRP, 128).T.copy(),
            DEG0=deg0.reshape(NGRP, 128).T.copy(),
            CT=CT,
            X0FM=x0_fm,
            E0FMH=e0fm_hi,
            STEPS=steps,
            STEPSR=stepsr,
        )
        for nm, wd in weights.items():
            m[f"{nm}_src16"] = wd["src"].astype(np.float16).reshape(128, -1)
            m[f"{nm}_dst16"] = wd["dst"].astype(np.float16).reshape(128, -1)
            m[f"{nm}_eh"] = wd["e"].astype(np.float16).reshape(128, -1)
        for nm in ["b1", "b2", "b3", "b3p", "bs1", "bss", "bs2", "b4", "bl"]:
            b = np.asarray(inputs[nm], np.float32)
            m[nm] = np.tile(b.reshape(1, -1), (128, 1))
        for nm in ["wg1", "wg2"]:
            m[nm] = wtile(np.asarray(inputs[nm], np.float32)).reshape(128, -1)
        for i, nm in enumerate(["ag1", "ag2"]):
            a = np.asarray(inputs[nm], np.float32)
            bgv = float(np.asarray(inputs["bg1" if i == 0 else "bg2"], np.float32)[0])
            m[nm] = np.tile(np.array([[a[0], a[1], a[0] + a[1], bgv]], np.float32), (128, 1))
        m["wl"] = np.asarray(inputs["wl"], np.float32)
        in_maps.append(m)

    return meta, in_maps, weights


def build(meta, weights):
    GC, EC, NCHUNK = meta["GC"], meta["EC"], meta["NCHUNK"]
    gsz = GC * 128
    HB = NGRP // 2          # groups per gather call (2 calls/round)
    hsz = HB * gsz          # idxs per gather call

    nc = bacc.Bacc(None, target_bir_lowering=False)

    P = {}

    def param(name, shape, dtype=F32):
        P[name] = nc.declare_dram_parameter(name, list(shape), dtype, isOutput=False)
        return P[name]

    param("S", [128, NCHUNK, 128], F16)
    param("ST", [128, NCHUNK, 128], F16)
    param("SRCF", [128, EC // 16], I16)
    param("PMASK", [128, NCHUNK], F32)
    param("RECIP0", [128, NGRP], F32)
    param("DEG0", [128, NGRP], F32)
    param("CT", [128, NGRP, 64, 128], F16)
    param("X0FM", [128, NGRP * 128], F32)
    param("E0FMH", [128, EC], F16)
    param("STEPS", [128, TK_LEVELS], F32)
    param("STEPSR", [1, TK_LEVELS * 128], F32)
    for nm, wd in weights.items():
        KTx = wd["src"].shape[1]
        KTe = wd["e"].shape[1]
        out = wd["src"].shape[2]
        param(f"{nm}_src16", [128, KTx * out], F16)
        param(f"{nm}_dst16", [128, KTx * out], F16)
        param(f"{nm}_eh", [128, KTe * out], F16)
    for nm, dd in [("b1", 128), ("b2", 128), ("b3", 128), ("b3p", 256),
                   ("bs1", 256), ("bss", 256), ("bs2", 128), ("b4", 128), ("bl", 4)]:
        param(nm, [128, dd], F32)
    param("wg1", [128, 2], F32)
    param("wg2", [128, 2], F32)
    param("ag1", [128, 4], F32)
    param("ag2", [128, 4], F32)
    param("wl", [128, 4], F32)

    OUT = nc.declare_dram_parameter("out", [128, NGRP, 4], F32, isOutput=True)

    LB = {nm: dict(wd) for nm, wd in weights.items()}

    with tile.TileContext(nc) as tc:
        psum = tc.alloc_tile_pool(name="ps", bufs=1, space="PSUM")
        persist = tc.alloc_tile_pool(name="persist", bufs=1)
        epool = tc.alloc_tile_pool(name="epool", bufs=1)
        dram = tc.alloc_tile_pool(name="dram", bufs=1, space="DRAM")

        gat_sem = nc.alloc_semaphore("gat_dma")

        def pload(name, shape, dtype=F32):
            t = persist.tile(list(shape), dtype, tag=name, name=name)
            nc.sync.dma_start(t[:], P[name][:])
            return t

        S_sb = pload("S", [128, NCHUNK, 128], F16)
        SRCF_sb = pload("SRCF", [128, EC // 16], I16)
        PMASK_sb = pload("PMASK", [128, NCHUNK], F32)
        RECIP0_sb = pload("RECIP0", [128, NGRP], F32)
        DEG0_sb = pload("DEG0", [128, NGRP], F32)
        STEPS_sb = pload("STEPS", [128, TK_LEVELS], F32)
        STEPSR_sb = pload("STEPSR", [1, TK_LEVELS * 128], F32)
        ag1_sb = pload("ag1", [128, 4], F32)
        ag2_sb = pload("ag2", [128, 4], F32)

        dum16 = persist.tile([1, 64], F16, tag="dum16", name="dum16")
        dum32 = persist.tile([1, 64], F32, tag="dum32", name="dum32")
        dumb = persist.tile([128, 64], F32, tag="dumb", name="dumb")
        ident = persist.tile([128, 128], F32, tag="ident", name="ident")
        make_identity(nc, ident[:])
        ident16 = persist.tile([128, 128], F16, tag="ident16", name="ident16")
        nc.vector.tensor_copy(ident16[:], ident[:])

        # node state: name -> dict(dram=[128, NGRP*F] f32 DRAM tile, F)
        state = {"x0": dict(dram=None, F=128)}
        # e-state: name -> dict(F, KT, kind, sb=tile|None, mir=dram|None)
        estate = {"e0": dict(F=128, KT=1, kind="host")}

        recips = {0: RECIP0_sb}
        degs = {0: DEG0_sb}
        astash = {}
        poolkeep = {}
        keeps = {0: None}
        pmasks = {0: PMASK_sb}   # [128, NCHUNK] masks for pool `we` weighting

        def new_estate(name, F):
            KT = F // 128
            kind = EPLACE[name]
            d = dict(F=F, KT=KT, kind=kind, sb=None, mir=None)
            if kind in ("sbuf", "sbufm"):
                d["sb"] = epool.tile([128, KT, NCHUNK, 128], F16,
                                     tag=ESLOT[name], name=f"esb_{name}")
            if kind in ("sbufm", "dram"):
                d["mir"] = dram.tile([128, KT, NCHUNK, 128], F16,
                                     tag=f"mir_{name}", name=f"mir_{name}")
            estate[name] = d
            return d

        def new_x(name, F):
            t = dram.tile([128, NGRP * F], F32, tag=f"x_{name}", name=f"x_{name}")
            state[name] = dict(dram=t, F=F)
            return t

        def load_nm(pool, xname, tag):
            st = state[xname]
            t = pool.tile([128, NGRP, st["F"]], F32, tag=tag, name=tag)
            nc.sync.dma_start(t[:], st["dram"][:].rearrange(
                "p (g f) -> p g f", g=NGRP, f=st["F"]))
            return t

        def load_fm(pool, xname, dtype, tag):
            """DRAM x_nm -> feature-major [128, KT, NGRP, 128] via PE transpose."""
            st = state[xname]
            F = st["F"]
            if xname == "x0":
                fm = pool.tile([128, 1, NGRP, 128], dtype, tag=tag, name=tag)
                if dtype == F32:
                    nc.sync.dma_start(fm[:], P["X0FM"][:])
                else:
                    tmp = pool.tile([128, 1, NGRP, 128], F32, tag=tag + "_t", name=tag + "_t")
                    nc.sync.dma_start(tmp[:], P["X0FM"][:])
                    nc.vector.tensor_copy(fm[:], tmp[:])
                return fm
            KT = F // 128
            xnm = load_nm(pool, xname, tag + "_nm")
            fm = pool.tile([128, KT, NGRP, 128], dtype, tag=tag, name=tag)
            for g in range(NGRP):
                for kt in range(KT):
                    tg = "ps_trA" if (g * KT + kt) % 2 == 0 else "ps_trB"
                    pt = psum.tile([128, 128], F32, tag=tg, name=tg)
                    nc.tensor.transpose(pt[:], xnm[:, g, kt * 128:(kt + 1) * 128], ident[:])
                    nc.vector.tensor_copy(fm[:, kt, g, :], pt[:])
            return fm

        def reload_S():
            nc.sync.dma_start(S_sb[:], P["S"][:])

        # ---------------- conv layer ----------------
        def conv_layer(li, name, x_parts, e_parts, wname, out, regime, edge_out,
                       want_deg, keepreg=None):
            wd = LB[wname]
            KTx = wd["src"].shape[1]
            KTe = wd["e"].shape[1]
            outP = out + 1 if want_deg else out
            wide = out + 128 if keepreg else out

            lp = tc.alloc_tile_pool(name=f"L{li}", bufs=1)
            lps = tc.alloc_tile_pool(name=f"L{li}d", bufs=2)
            lps1 = tc.alloc_tile_pool(name=f"L{li}s", bufs=1)

            w16s = lp.tile([128, KTx, out], F16, tag="w16s", name="w16s")
            nc.sync.dma_start(w16s[:], P[f"{wname}_src16"][:])
            w16d = lp.tile([128, KTx, out], F16, tag="w16d", name="w16d")
            nc.sync.dma_start(w16d[:], P[f"{wname}_dst16"][:])
            weh = lp.tile([128, KTe, out], F16, tag="weh", name="weh")
            nc.sync.dma_start(weh[:], P[f"{wname}_eh"][:])
            brep = lp.tile([128, out], F32, tag="brep", name="brep")
            nc.sync.dma_start(brep[:], P[BNAME[wname]][:])

            # --- node-side H tables (per-group x loads) ---
            x0fm16 = None
            if "x0" in x_parts:
                x0t = lp.tile([128, NGRP * 128], F32, tag="x0t", name="x0t")
                nc.sync.dma_start(x0t[:], P["X0FM"][:])
                x0fm16 = lp.tile([128, NGRP * 128], F16, tag="x0f16", name="x0f16")
                nc.vector.tensor_copy(x0fm16[:], x0t[:])
            hsrc_sb = lp.tile([128, NGRP, out], F16, tag="hsrc_sb", name="hsrc_sb")
            hdst_sb = lp.tile([128, NGRP, out], F16, tag="hdst_sb", name="hdst_sb")
            for g in range(NGRP):
                fmg = []
                for xi, xp_ in enumerate(x_parts):
                    if xp_ == "x0":
                        fmg.append(("x0",))
                        continue
                    F = state[xp_]["F"]
                    KTp = F // 128
                    xg = lps.tile([128, 256], F32, tag=f"xg{xi}", name=f"xg{xi}")
                    nc.sync.dma_start(xg[:, :F],
                                      state[xp_]["dram"][:, g * F:(g + 1) * F])
                    fg = lps.tile([128, 2, 128], F16, tag=f"fg{xi}", name=f"fg{xi}")
                    for kt in range(KTp):
                        tg = "ps_trA" if kt % 2 == 0 else "ps_trB"
                        pt = psum.tile([128, 128], F32, tag=tg, name=tg)
                        nc.tensor.transpose(pt[:], xg[:, kt * 128:(kt + 1) * 128],
                                            ident[:])
                        nc.vector.tensor_copy(fg[:, kt, :], pt[:])
                    fmg.append(("t", fg))

                def fm_ap(xi, kt):
                    if fmg[xi][0] == "x0":
                        return x0fm16[:, g * 128:(g + 1) * 128]
                    return fmg[xi][1][:, kt, :]

                ps_s = psum.tile([128, out], F32, tag="ps_node", name="ps_node")
                ps_d = psum.tile([128, out], F32, tag="ps_node2", name="ps_node2")
                kt_glob = 0
                for xi, xp_ in enumerate(x_parts):
                    KTp = state[xp_]["F"] // 128
                    for kt in range(KTp):
                        last = (kt_glob == KTx - 1)
                        nc.tensor.matmul(ps_s[:], fm_ap(xi, kt), w16s[:, kt_glob, :],
                                         start=(kt_glob == 0), stop=last)
                        nc.tensor.matmul(ps_d[:], fm_ap(xi, kt), w16d[:, kt_glob, :],
                                         start=(kt_glob == 0), stop=last)
                        kt_glob += 1
                nc.vector.tensor_copy(hsrc_sb[:, g, :], ps_s[:])
                t1 = lps1.tile([128, out], F32, tag="hdtmp", name="hdtmp")
                nc.vector.tensor_tensor(t1[:], ps_d[:], brep[:], op=OP.add)
                nc.vector.tensor_copy(hdst_sb[:, g, :], t1[:])

            # --- AllGather Hsrc (optionally with pool-keep column) ---
            ag_in = dram.tile([NLOC, wide], F16, tag="ag_in", name="ag_in")
            ag_out = dram.tile([N, wide], F16, tag="ag_out", name="ag_out",
                               addr_space="Shared")
            agv_in = ag_in[:].rearrange("(g p) f -> p g f", p=128, g=NGRP)
            nc.sync.dma_start(agv_in[:, :, :out], hsrc_sb[:])
            if keepreg:
                nc.sync.dma_start(agv_in[:, :, out:out + 1],
                                  poolkeep[keepreg][:].unsqueeze(2))
            nc.gpsimd.collective_compute(
                "AllGather", OP.bypass, replica_groups=[list(range(NCORES))],
                ins=[ag_in[:].opt()], outs=[ag_out[:].opt()])

            # --- gathers: prepare early, trigger after AllGather ---
            hgs = []
            for h in range(NGRP):
                hg = lp.tile([128, GC, wide], F16, tag=f"hg{h % 2}",
                             name=f"hg{h % 2}")
                nc.gpsimd.dma_gather(
                    out_ap=hg[:], in_ap=ag_out[:],
                    idxs_ap=SRCF_sb[:, h * (gsz // 16):(h + 1) * (gsz // 16)],
                    num_idxs=gsz, num_idxs_reg=gsz, elem_size=wide,
                    single_packet=False)
                hgs.append(hg)

            if edge_out:
                eo = new_estate(LAYER_EOUT[name], out)
            aggsb = lp.tile([128, NGRP, outP], F16, tag="aggsb", name="aggsb")
            fuse = FUSESPEC.get(name)
            if fuse:
                stash = persist.tile([128, NGRP, out], F16, tag=f"ast_{fuse}",
                                     name=f"ast_{fuse}")
                astash[fuse] = stash

            # --- edge phase ---
            for g in range(NGRP):
                # edge-feature (feature-major) inputs for this group
                efm_slices = []   # list of (tile, index-fn) per e_part kt
                for ei, ep in enumerate(e_parts):
                    ed = estate[ep]
                    if ed["kind"] == "host":
                        t = lps.tile([128, gsz], F16, tag=f"efm{ei}h", name=f"efm{ei}h")
                        nc.sync.dma_start(t[:], P["E0FMH"][:, g * gsz:(g + 1) * gsz])
                        efm_slices.append(("host", t))
                    elif ECONSUME[(name, ep)] == "sbuf":
                        efm_slices.append(("sbuf", ed["sb"], ed["KT"]))
                    else:
                        t = lps.tile([128, ed["KT"], GC, 128], F16,
                                     tag=f"efm{ei}s", name=f"efm{ei}s")
                        nc.sync.dma_start(
                            t[:], ed["mir"][:, :, g * GC:(g + 1) * GC, :])
                        efm_slices.append(("stream", t, ed["KT"]))

                def efm_ap(t_in_g, kt_glob):
                    k = kt_glob
                    for es in efm_slices:
                        if es[0] == "host":
                            if k == 0:
                                return es[1][:, t_in_g * 128:(t_in_g + 1) * 128]
                            k -= 1
                        elif es[0] == "sbuf":
                            if k < es[2]:
                                return es[1][:, k, g * GC + t_in_g, :]
                            k -= es[2]
                        else:
                            if k < es[2]:
                                return es[1][:, k, t_in_g, :]
                            k -= es[2]
                    raise AssertionError

                spool = lps if li >= 4 else lps1
                stg = spool.tile([128, GC, 128], F16, tag="stg", name="stg")
                nc.sync.dma_start(stg[:], P["ST"][:, g * GC:(g + 1) * GC, :])

                mgo = lps.tile([128, GC, outP], F16, tag="mgo", name="mgo")
                if want_deg:
                    nc.vector.memset(mgo[:, :, out:outP], 1.0)
                if fuse:
                    ego = lps1.tile([128, GC, out], F16, tag="ego", name="ego")

                if edge_out and eo["kind"] == "dram":
                    fmstage = lps1.tile([128, eo["KT"], GC, 128], F16,
                                       tag="fmstage", name="fmstage")

                hgt = hgs[g]
                if keepreg:
                    if g == 0:
                        sm16 = persist.tile([128, NCHUNK], F16,
                                            tag=f"smask16_{keepreg}",
                                            name=f"smask16_{keepreg}")
                        pmasks[keepreg] = sm16
                    nc.vector.tensor_tensor(sm16[:, g * GC:(g + 1) * GC],
                                            hgt[:, :, out],
                                            PMASK_sb[:, g * GC:(g + 1) * GC],
                                            op=OP.mult)
                    nc.vector.tensor_tensor(
                        S_sb[:, g * GC:(g + 1) * GC, :],
                        S_sb[:, g * GC:(g + 1) * GC, :],
                        sm16[:, g * GC:(g + 1) * GC].unsqueeze(2).to_broadcast(
                            [128, GC, 128]),
                        op=OP.mult)
                ps_agg = psum.tile([128, outP], F32, tag="ps_agg", name="ps_agg")
                if fuse:
                    a_ps = psum.tile([128, out], F32, tag="ps_node", name="ps_node")
                for t in range(GC):
                    c = g * GC + t
                    ptag = "ps_msgA" if t % 2 == 0 else "ps_msgB"
                    pm = psum.tile([128, out], F32, tag=ptag, name=ptag)
                    for kt in range(KTe):
                        nc.tensor.matmul(pm[:], efm_ap(t, kt),
                                         weh[:, kt, :], start=(kt == 0), stop=False)
                    nc.tensor.matmul(pm[:], stg[:, t, :], hdst_sb[:, g, :],
                                     start=False, stop=True)
                    nc.vector.tensor_tensor(mgo[:, t, :out], pm[:],
                                            hgt[:, t, :out], op=OP.add)
                    if edge_out:
                        for kt in range(out // 128):
                            tg = "ps_trA" if (t * 2 + kt) % 2 == 0 else "ps_trB"
                            pt = psum.tile([128, 128], F16, tag=tg, name=tg)
                            nc.tensor.transpose(
                                pt[:], mgo[:, t, kt * 128:(kt + 1) * 128], ident16[:])
                            dst_ap = (fmstage[:, kt, t, :]
                                      if eo["kind"] == "dram"
                                      else eo["sb"][:, kt, c, :])
                            nc.scalar.activation(dst_ap, pt[:], AF.Relu)
                    nc.tensor.matmul(ps_agg[:], S_sb[:, c, :], mgo[:, t, :],
                                     start=(t == 0), stop=(t == GC - 1))
                    if fuse:
                        nc.scalar.activation(ego[:, t, :], mgo[:, t, :out], AF.Relu)
                        nc.tensor.matmul(a_ps[:], S_sb[:, c, :], ego[:, t, :],
                                         start=(t == 0), stop=(t == GC - 1))
                nc.vector.tensor_copy(aggsb[:, g, :], ps_agg[:])
                if fuse:
                    nc.vector.tensor_copy(stash[:, g, :], a_ps[:])
                if edge_out:
                    if eo["kind"] == "dram":
                        nc.sync.dma_start(
                            eo["mir"][:, :, g * GC:(g + 1) * GC, :], fmstage[:])
                    elif eo["kind"] == "sbufm":
                        nc.sync.dma_start(
                            eo["mir"][:, :, g * GC:(g + 1) * GC, :],
                            eo["sb"][:, :, g * GC:(g + 1) * GC, :])

            # --- node update ---
            xout = LAYER_XOUT[name]
            if want_deg:
                dsum = persist.tile([128, NGRP], F32, tag=f"degsum{regime}",
                                    name=f"degsum{regime}")
                nc.vector.tensor_copy(dsum[:], aggsb[:, :, out])
                kp = keeps[regime]
                ddt = lp.tile([128, NGRP], F32, tag="ddt", name="ddt")
                nc.vector.tensor_tensor(ddt[:], dsum[:], kp[:], op=OP.mult)
                nc.vector.tensor_scalar(ddt[:], ddt[:], 1.0, None, op0=OP.max)
                rec = persist.tile([128, NGRP], F32, tag=f"recip{regime}",
                                   name=f"recip{regime}")
                nc.vector.reciprocal(rec[:], ddt[:])
                recips[regime] = rec
                degs[regime] = dsum
            rec = recips[regime]
            kp = keeps[regime]
            fac = lp.tile([128, NGRP], F32, tag="fac", name="fac")
            if kp is not None:
                nc.vector.tensor_tensor(fac[:], rec[:], kp[:], op=OP.mult)
            else:
                nc.vector.tensor_copy(fac[:], rec[:])

            xtmp = lp.tile([128, NGRP, out], F32, tag="xtmp", name="xtmp")
            nc.vector.tensor_tensor(xtmp[:], aggsb[:, :, :out],
                                    fac[:].unsqueeze(2).to_broadcast([128, NGRP, out]),
                                    op=OP.mult)
            nc.scalar.activation(xtmp[:], xtmp[:], AF.Relu)
            xd = new_x(xout, out)
            nc.sync.dma_start(xd[:].rearrange("p (g f) -> p g f", g=NGRP, f=out), xtmp[:])

            lps1.release()
            lps.release()
            lp.release()

        # ---------------- term layer (aggregation only, no gather) ----------
        def term_layer(li, name, x_parts, wname, out, regime):
            wd = LB[wname]
            KTx = wd["src"].shape[1]
            KTe = wd["e"].shape[1]
            aparts = TERMSPEC[name]["aparts"]

            lp = tc.alloc_tile_pool(name=f"T{li}", bufs=1)
            lps = tc.alloc_tile_pool(name=f"T{li}d", bufs=2)
            lps1 = tc.alloc_tile_pool(name=f"T{li}s", bufs=1)

            w16s = lp.tile([128, KTx, out], F16, tag="w16s", name="w16s")
            nc.sync.dma_start(w16s[:], P[f"{wname}_src16"][:])
            w16d = lp.tile([128, KTx, out], F16, tag="w16d", name="w16d")
            nc.sync.dma_start(w16d[:], P[f"{wname}_dst16"][:])
            weh = lp.tile([128, KTe, out], F16, tag="weh", name="weh")
            nc.sync.dma_start(weh[:], P[f"{wname}_eh"][:])
            brep = lp.tile([128, out], F32, tag="brep", name="brep")
            nc.sync.dma_start(brep[:], P[BNAME[wname]][:])

            hsrc_sb = lp.tile([128, NGRP, out], F16, tag="hsrc_sb", name="hsrc_sb")
            hdst_sb = lp.tile([128, NGRP, out], F16, tag="hdst_sb", name="hdst_sb")
            for g in range(NGRP):
                fmg = []
                for xi, xp_ in enumerate(x_parts):
                    F = state[xp_]["F"]
                    KTp = F // 128
                    xg = lps.tile([128, 256], F32, tag=f"xg{xi}", name=f"xg{xi}")
                    nc.sync.dma_start(xg[:, :F],
                                      state[xp_]["dram"][:, g * F:(g + 1) * F])
                    fg = lps.tile([128, 2, 128], F16, tag=f"fg{xi}", name=f"fg{xi}")
                    for kt in range(KTp):
                        tg = "ps_trA" if kt % 2 == 0 else "ps_trB"
                        pt = psum.tile([128, 128], F32, tag=tg, name=tg)
                        nc.tensor.transpose(pt[:], xg[:, kt * 128:(kt + 1) * 128],
                                            ident[:])
                        nc.vector.tensor_copy(fg[:, kt, :], pt[:])
                    fmg.append(fg)
                ps_s = psum.tile([128, out], F32, tag="ps_node", name="ps_node")
                ps_d = psum.tile([128, out], F32, tag="ps_node2", name="ps_node2")
                kt_glob = 0
                for xi, xp_ in enumerate(x_parts):
                    KTp = state[xp_]["F"] // 128
                    for kt in range(KTp):
                        last = (kt_glob == KTx - 1)
                        nc.tensor.matmul(ps_s[:], fmg[xi][:, kt, :], w16s[:, kt_glob, :],
                                         start=(kt_glob == 0), stop=last)
                        nc.tensor.matmul(ps_d[:], fmg[xi][:, kt, :], w16d[:, kt_glob, :],
                                         start=(kt_glob == 0), stop=last)
                        kt_glob += 1
                nc.vector.tensor_copy(hsrc_sb[:, g, :], ps_s[:])
                t1 = lps1.tile([128, out], F32, tag="hdtmp", name="hdtmp")
                nc.vector.tensor_tensor(t1[:], ps_d[:], brep[:], op=OP.add)
                nc.vector.tensor_copy(hdst_sb[:, g, :], t1[:])

            ag_in = dram.tile([NLOC, out], F16, tag="ag_in", name="ag_in")
            ag_out = dram.tile([N, out], F16, tag="ag_out", name="ag_out",
                               addr_space="Shared")
            nc.sync.dma_start(ag_in[:].rearrange("(g p) f -> p g f", p=128, g=NGRP),
                              hsrc_sb[:])
            nc.gpsimd.collective_compute(
                "AllGather", OP.bypass, replica_groups=[list(range(NCORES))],
                ins=[ag_in[:].opt()], outs=[ag_out[:].opt()])

            tbl = lp.tile([128, 64, out], F16, tag="tbl", name="tbl")
            nc.sync.dma_start(tbl[:],
                              ag_out[:].rearrange("(c p) f -> p c f", c=64, p=128))

            degt = degs[regime]
            aggsb = lp.tile([128, NGRP, out], F32, tag="aggsb", name="aggsb")
            for g in range(NGRP):
                ctg = lps.tile([128, 64, 128], F16, tag="ctg", name="ctg")
                nc.sync.dma_start(ctg[:], P["CT"][:, g, :, :])
                ats = []
                for ai, (anm, woff, KTa) in enumerate(aparts):
                    st_t = astash[anm]
                    at = lps.tile([128, 2, 128], F16, tag=f"at{ai}", name=f"at{ai}")
                    for kt in range(KTa):
                        tg = "ps_trA" if kt % 2 == 0 else "ps_trB"
                        pt = psum.tile([128, 128], F16, tag=tg, name=tg)
                        nc.tensor.transpose(pt[:], st_t[:, g, kt * 128:(kt + 1) * 128],
                                            ident16[:])
                        nc.vector.tensor_copy(at[:, kt, :], pt[:])
                    ats.append((at, woff, KTa))
                ps_a = psum.tile([128, out], F32, tag="ps_agg", name="ps_agg")
                for k in range(64):
                    nc.tensor.matmul(ps_a[:], ctg[:, k, :], tbl[:, k, :],
                                     start=(k == 0), stop=False)
                nmm = sum(KTa for _, _, KTa in aparts)
                j = 0
                for at, woff, KTa in ats:
                    for kt in range(KTa):
                        j += 1
                        nc.tensor.matmul(ps_a[:], at[:, kt, :], weh[:, woff + kt, :],
                                         start=False, stop=(j == nmm))
                t2 = lps1.tile([128, out], F32, tag="t2g", name="t2g")
                nc.vector.tensor_scalar(t2[:], hdst_sb[:, g, :], degt[:, g:g + 1],
                                        None, op0=OP.mult)
                nc.vector.tensor_tensor(aggsb[:, g, :], ps_a[:], t2[:], op=OP.add)

            xout = LAYER_XOUT[name]
            rec = recips[regime]
            kp = keeps[regime]
            fac = lp.tile([128, NGRP], F32, tag="fac", name="fac")
            if kp is not None:
                nc.vector.tensor_tensor(fac[:], rec[:], kp[:], op=OP.mult)
            else:
                nc.vector.tensor_copy(fac[:], rec[:])
            xtmp = lp.tile([128, NGRP, out], F32, tag="xtmp", name="xtmp")
            nc.vector.tensor_tensor(xtmp[:], aggsb[:],
                                    fac[:].unsqueeze(2).to_broadcast([128, NGRP, out]),
                                    op=OP.mult)
            nc.scalar.activation(xtmp[:], xtmp[:], AF.Relu)
            xd = new_x(xout, out)
            nc.sync.dma_start(xd[:].rearrange("p (g f) -> p g f", g=NGRP, f=out), xtmp[:])
            lps1.release()
            lps.release()
            lp.release()

        # ---------------- GAT pool ----------------
        def gat_pool(pi, xname, wgname, agname, k, regime_in, regime_out, xpout):
            pp = tc.alloc_tile_pool(name=f"P{pi}", bufs=1)
            st = state[xname]
            Fdim = st["F"]
            KT = Fdim // 128
            agv = ag1_sb if pi == 0 else ag2_sb

            wg = pp.tile([128, KT], F32, tag="wg", name="wg")
            nc.sync.dma_start(wg[:], P[wgname][:])
            wg16 = pp.tile([128, KT], F16, tag="wg16", name="wg16")
            nc.vector.tensor_copy(wg16[:], wg[:])

            hs = pp.tile([128, NGRP], F32, tag="hs", name="hs")
            for g in range(NGRP):
                xg = pp.tile([128, 256], F32, tag="gxg", name="gxg")
                nc.sync.dma_start(xg[:, :Fdim],
                                  state[xname]["dram"][:, g * Fdim:(g + 1) * Fdim])
                fg = pp.tile([128, 2, 128], F16, tag="gfg", name="gfg")
                for kt in range(KT):
                    tg = "ps_trA" if kt % 2 == 0 else "ps_trB"
                    pt = psum.tile([128, 128], F32, tag=tg, name=tg)
                    nc.tensor.transpose(pt[:], xg[:, kt * 128:(kt + 1) * 128], ident[:])
                    nc.vector.tensor_copy(fg[:, kt, :], pt[:])
                ph = psum.tile([128, 1], F32, tag="ps_small", name="ps_small")
                for kt in range(KT):
                    nc.tensor.matmul(ph[:], fg[:, kt, :], wg16[:, kt:kt + 1],
                                     start=(kt == 0), stop=(kt == KT - 1))
                nc.vector.tensor_copy(hs[:, g:g + 1], ph[:])
            hs16 = pp.tile([128, NGRP], F16, tag="hs16", name="hs16")
            nc.vector.tensor_copy(hs16[:], hs[:])

            hrep = pp.tile([128, NGRP, 64], F32, tag="hrep", name="hrep")
            nc.vector.tensor_copy(hrep[:], hs[:].unsqueeze(2).to_broadcast([128, NGRP, 64]))
            hs_in = dram.tile([NLOC, 64], F32, tag="hs_ag_in", name="hs_ag_in")
            hs_out = dram.tile([N, 64], F32, tag="hs_ag_out", name="hs_ag_out",
                               addr_space="Shared")
            nc.sync.dma_start(hs_in[:].rearrange("(g p) f -> p g f", p=128, g=NGRP), hrep[:])
            nc.gpsimd.collective_compute(
                "AllGather", OP.bypass, replica_groups=[list(range(NCORES))],
                ins=[hs_in[:].opt()], outs=[hs_out[:].opt()])

            # per-edge hs[src] via prepared gather
            gbig = pp.tile([128, NCHUNK, 64], F32, tag="gbig", name="gbig")
            for h in range(2):
                nc.gpsimd.dma_gather(
                    out_ap=gbig[:, h * (NCHUNK // 2):(h + 1) * (NCHUNK // 2), :],
                    in_ap=hs_out[:],
                    idxs_ap=SRCF_sb[:, h * (hsz // 16):(h + 1) * (hsz // 16)],
                    num_idxs=hsz, num_idxs_reg=hsz, elem_size=64,
                    single_packet=False)
            hsrc_e = pp.tile([128, NCHUNK], F32, tag="hsrc_e", name="hsrc_e")
            nc.vector.tensor_copy(hsrc_e[:], gbig[:, :, 0])

            # per-edge hs[dst] via ST matmul (dst local)
            hdst_e = pp.tile([128, NCHUNK], F32, tag="hdst_e", name="hdst_e")
            for g in range(NGRP):
                stg = pp.tile([128, GC, 128], F16, tag="stgp", name="stgp")
                nc.sync.dma_start(stg[:], P["ST"][:, g * GC:(g + 1) * GC, :])
                pg = psum.tile([128, GC], F32, tag="ps_small", name="ps_small")
                for t in range(GC):
                    nc.tensor.matmul(pg[:, t:t + 1], stg[:, t, :], hs16[:, g:g + 1],
                                     start=True, stop=True)
                nc.vector.tensor_copy(hdst_e[:, g * GC:(g + 1) * GC], pg[:])

            u = pp.tile([128, NCHUNK], F32, tag="u", name="u")
            nc.vector.tensor_scalar(hdst_e[:], hdst_e[:], agv[:, 1:2], None, op0=OP.mult)
            nc.vector.scalar_tensor_tensor(u[:], hsrc_e[:], agv[:, 0:1], hdst_e[:],
                                           op0=OP.mult, op1=OP.add)
            lg = pp.tile([128, NCHUNK], F32, tag="lg", name="lg")
            nc.vector.tensor_scalar(lg[:], u[:], LRELU, None, op0=OP.mult)
            nc.vector.tensor_tensor(lg[:], lg[:], u[:], op=OP.max)
            w = pp.tile([128, NCHUNK], F32, tag="w", name="w")
            nc.scalar.activation(w[:], lg[:], AF.Exp)
            we = pp.tile([128, NCHUNK], F32, tag="we", name="we")
            nc.vector.tensor_tensor(we[:], w[:], pmasks[regime_in][:], op=OP.mult)
            wpair = pp.tile([128, NCHUNK, 2], F16, tag="wpair", name="wpair")
            nc.vector.tensor_copy(wpair[:, :, 0], we[:])
            nc.vector.tensor_tensor(wpair[:, :, 1], we[:], hsrc_e[:], op=OP.mult)

            gsums = pp.tile([128, NGRP, 2], F32, tag="gsums", name="gsums")
            for g in range(NGRP):
                pg = psum.tile([128, 2], F32, tag="ps_small", name="ps_small")
                for t in range(GC):
                    c = g * GC + t
                    nc.tensor.matmul(pg[:], S_sb[:, c, :], wpair[:, c, :],
                                     start=(t == 0), stop=(t == GC - 1))
                nc.vector.tensor_copy(gsums[:, g, :], pg[:])

            us = pp.tile([128, NGRP], F32, tag="us", name="us")
            nc.vector.tensor_scalar(us[:], hs[:], agv[:, 2:3], None, op0=OP.mult)
            ls = pp.tile([128, NGRP], F32, tag="ls", name="ls")
            nc.vector.tensor_scalar(ls[:], us[:], LRELU, None, op0=OP.mult)
            nc.vector.tensor_tensor(ls[:], ls[:], us[:], op=OP.max)
            exs = pp.tile([128, NGRP], F32, tag="exs", name="exs")
            nc.scalar.activation(exs[:], ls[:], AF.Exp)

            kp_in = keeps[regime_in]
            den = pp.tile([128, NGRP], F32, tag="den", name="den")
            num = pp.tile([128, NGRP], F32, tag="num", name="num")
            if kp_in is not None:
                nc.vector.tensor_tensor(den[:], gsums[:, :, 0], kp_in[:], op=OP.mult)
                nc.vector.tensor_tensor(num[:], gsums[:, :, 1], kp_in[:], op=OP.mult)
            else:
                nc.vector.tensor_copy(den[:], gsums[:, :, 0])
                nc.vector.tensor_copy(num[:], gsums[:, :, 1])
            nc.vector.tensor_tensor(den[:], den[:], exs[:], op=OP.add)
            t2 = pp.tile([128, NGRP], F32, tag="t2", name="t2")
            nc.vector.tensor_tensor(t2[:], exs[:], hs[:], op=OP.mult)
            nc.vector.tensor_tensor(num[:], num[:], t2[:], op=OP.add)
            rden = pp.tile([128, NGRP], F32, tag="rden", name="rden")
            nc.vector.reciprocal(rden[:], den[:])
            score = pp.tile([128, NGRP], F32, tag="score", name="score")
            nc.vector.tensor_tensor(score[:], num[:], rden[:], op=OP.mult)
            nc.vector.tensor_scalar(score[:], score[:], agv[:, 3:4], None, op0=OP.add)

            sc_in = dram.tile([NLOC, 1], F32, tag="sc_ag_in", name="sc_ag_in")
            sc_out = dram.tile([N, 1], F32, tag="sc_ag_out", name="sc_ag_out",
                               addr_space="Shared")
            nc.sync.dma_start(sc_in[:].rearrange("(g p) f -> p g f", p=128, g=NGRP),
                              score[:].unsqueeze(2))
            nc.gpsimd.collective_compute(
                "AllGather", OP.bypass, replica_groups=[list(range(NCORES))],
                ins=[sc_in[:].opt()], outs=[sc_out[:].opt()])

            s_full = pp.tile([128, 64], F32, tag="s_full", name="s_full")
            nc.sync.dma_start(s_full[:], sc_out[:].rearrange("(p c) f -> p (c f)",
                                                             p=128, c=64))
            # masked local / full scores
            if pi == 1:
                k1f = keeps["full1"]
                m1 = pp.tile([128, 64], F32, tag="m1", name="m1")
                nc.vector.tensor_tensor(m1[:], s_full[:], k1f[:], op=OP.mult)
                m2 = pp.tile([128, 64], F32, tag="m2", name="m2")
                nc.vector.tensor_scalar(m2[:], k1f[:], 1.0, None, op0=OP.subtract)
                nc.vector.tensor_scalar(m2[:], m2[:], 1e30, None, op0=OP.mult)
                nc.vector.tensor_tensor(s_full[:], m1[:], m2[:], op=OP.add)
                sl1 = pp.tile([128, NGRP], F32, tag="sl1", name="sl1")
                nc.vector.tensor_tensor(sl1[:], score[:], kp_in[:], op=OP.mult)
                sl2 = pp.tile([128, NGRP], F32, tag="sl2", name="sl2")
                nc.vector.tensor_scalar(sl2[:], kp_in[:], 1.0, None, op0=OP.subtract)
                nc.vector.tensor_scalar(sl2[:], sl2[:], 1e30, None, op0=OP.mult)
                score_m = pp.tile([128, NGRP], F32, tag="score_m", name="score_m")
                nc.vector.tensor_tensor(score_m[:], sl1[:], sl2[:], op=OP.add)
            else:
                score_m = score

            # threshold refinement: counts via compact [128, 128, 64] compare
            tlo = pp.tile([1, 1], F32, tag="tlo0", name="tlo0")
            nc.vector.memset(tlo[:], TK_LO)
            for lvl in range(TK_LEVELS):
                wl = TK_RANGE / (128.0 ** (lvl + 1))
                T_row = pp.tile([1, 128], F32, tag=f"Tr{lvl}", name=f"Tr{lvl}")
                nc.vector.tensor_scalar(T_row[:],
                                        STEPSR_sb[0:1, lvl * 128:(lvl + 1) * 128],
                                        tlo[:], None, op0=OP.add)
                T_all = pp.tile([128, 128], F32, tag="T_all", name="T_all")
                nc.gpsimd.partition_broadcast(T_all[:], T_row[:])
                cmp3 = pp.tile([128, 128, 64], F16, tag="cmp3", name="cmp3")
                nc.vector.tensor_tensor(
                    cmp3[:], s_full[:].unsqueeze(1).to_broadcast([128, 128, 64]),
                    T_all[:].unsqueeze(2).to_broadcast([128, 128, 64]), op=OP.is_ge)
                cntp = pp.tile([128, 128], F32, tag="cntp", name="cntp")
                nc.vector.tensor_reduce(cntp[:].unsqueeze(2), cmp3[:],
                                        axis=mybir.AxisListType.X, op=OP.add)
                cnta = pp.tile([128, 128], F32, tag="cnta", name="cnta")
                nc.gpsimd.partition_all_reduce(cnta[:], cntp[:], channels=128,
                                               reduce_op=bass_isa.ReduceOp.add)
                cj = pp.tile([128, 128], F32, tag="cj", name="cj")
                nc.vector.tensor_scalar(cj[:], cnta[:], float(k), None, op0=OP.is_ge)
                sj = pp.tile([128, 1], F32, tag=f"sj{lvl}", name=f"sj{lvl}")
                nc.vector.tensor_reduce(sj[:], cj[:], axis=mybir.AxisListType.X,
                                        op=OP.add)
                tlo2 = pp.tile([1, 1], F32, tag=f"tlo{lvl + 1}", name=f"tlo{lvl + 1}")
                nc.vector.scalar_tensor_tensor(tlo2[:], sj[0:1, :], wl, tlo[:],
                                               op0=OP.mult, op1=OP.add)
                tlo = tlo2

            tlo_b = pp.tile([128, 1], F32, tag="tlob_f", name="tlob_f")
            nc.gpsimd.partition_broadcast(tlo_b[:], tlo[:])

            keep_full = persist.tile([128, 64], F32, tag=f"keep{pi}f", name=f"keep{pi}f")
            nc.vector.tensor_scalar(keep_full[:], s_full[:], tlo_b[:], None, op0=OP.is_ge)
            keep_loc = persist.tile([128, NGRP], F32, tag=f"keep{pi}l", name=f"keep{pi}l")
            nc.vector.tensor_scalar(keep_loc[:], score_m[:], tlo_b[:], None, op0=OP.is_ge)
            keeps["full1" if pi == 0 else "full2"] = keep_full
            keeps[regime_out] = keep_loc

            # keep vector for the next conv layer's merged gather column
            kl16 = persist.tile([128, NGRP], F16, tag=f"plk{regime_out}",
                                name=f"plk{regime_out}")
            nc.vector.tensor_copy(kl16[:], keep_loc[:])
            poolkeep[regime_out] = kl16

            # xp = x * (1 + keep*(tanh(score)-1))
            th = pp.tile([128, NGRP], F32, tag="th", name="th")
            nc.scalar.activation(th[:], score[:], AF.Tanh)
            nc.vector.tensor_scalar(th[:], th[:], 1.0, None, op0=OP.subtract)
            nc.vector.tensor_tensor(th[:], th[:], keep_loc[:], op=OP.mult)
            nc.vector.tensor_scalar(th[:], th[:], 1.0, None, op0=OP.add)
            xin = load_nm(pp, xname, "xin")
            nc.vector.tensor_tensor(xin[:], xin[:],
                                    th[:].unsqueeze(2).to_broadcast([128, NGRP, Fdim]),
                                    op=OP.mult)
            xd = new_x(xpout, Fdim)
            nc.sync.dma_start(xd[:].rearrange("p (g f) -> p g f", g=NGRP, f=Fdim), xin[:])
            pp.release()

        # ---------------- residual ----------------
        def residual(dst_name, a_name, b_name, keep, F):
            rp = tc.alloc_tile_pool(name=f"R{dst_name}", bufs=1)
            a = load_nm(rp, a_name, "ra")
            b = load_nm(rp, b_name, "rb")
            t = rp.tile([128, NGRP, F], F32, tag="rt", name="rt")
            nc.vector.tensor_tensor(t[:], b[:],
                                    keep[:].unsqueeze(2).to_broadcast([128, NGRP, F]),
                                    op=OP.mult)
            nc.vector.tensor_tensor(t[:], t[:], a[:], op=OP.add)
            xd = new_x(dst_name, F)
            nc.sync.dma_start(xd[:].rearrange("p (g f) -> p g f", g=NGRP, f=F), t[:])
            rp.release()

        # ---------------- network ----------------
        klayers = int(os.environ.get("KLAYERS", "99"))
        kpool = int(os.environ.get("KPOOL", "99"))
        smask1_16 = None
        last_x = "x0"
        for li, spec in enumerate(LAYERS):
            if li >= klayers:
                break
            name, x_parts, e_parts, wname, out, regime, edge_out, want_deg = spec
            if name == "conv4":
                reload_S()
            if name in TERMSPEC:
                term_layer(li, name, x_parts, wname, out, regime)
            else:
                kr = regime if name in ("convs1", "convss") else None
                conv_layer(li, name, x_parts, e_parts, wname, out, regime,
                           edge_out, want_deg, keepreg=kr)
            last_x = LAYER_XOUT[name]
            if name == "conv3p" and kpool >= 1:
                gat_pool(0, "xs1", "wg1", "ag1", N // 2, 0, 1, "xp")
                # no SBUF-resident e-states are used past this point
                epool.release()
                last_x = "xp"
            elif name == "convs1" and kpool >= 2:
                gat_pool(1, "xs2", "wg2", "ag2", N // 4, 1, 2, "xpp")
                last_x = "xpp"
            elif name == "convss2":
                residual("xs22", "xs2", "xss2", keeps[2], 256)
            elif name == "convs2":
                residual("x32", "x3", "xs3", keeps[1], 128)

        kdbg = os.environ.get("KDBG", "")

        # ---------------- final linear ----------------
        fp = tc.alloc_tile_pool(name="fin", bufs=1)
        wl_sb = fp.tile([128, 4], F32, tag="wl_sb", name="wl_sb")
        nc.sync.dma_start(wl_sb[:], P["wl"][:])
        bl_sb = fp.tile([128, 4], F32, tag="bl_sb", name="bl_sb")
        nc.sync.dma_start(bl_sb[:], P["bl"][:])
        x5fm = load_fm(fp, "x5", F16, "x5fm")
        wl16 = fp.tile([128, 4], F16, tag="wl16", name="wl16")
        nc.vector.tensor_copy(wl16[:], wl_sb[:])
        outsb = fp.tile([128, NGRP, 4], F32, tag="outsb", name="outsb")
        for g in range(NGRP):
            po = psum.tile([128, 4], F32, tag="ps_small", name="ps_small")
            nc.tensor.matmul(po[:], x5fm[:, 0, g, :], wl16[:], start=True, stop=True)
            nc.vector.tensor_tensor(outsb[:, g, :], po[:], bl_sb[:], op=OP.add)
        if kdbg:
            xd = state[kdbg]["dram"]
            Fd = state[kdbg]["F"]
            nc.sync.dma_start(OUT[:], xd[:].rearrange(
                "p (g f) -> p g f", g=NGRP, f=Fd)[:, :, 0:4])
        else:
            nc.sync.dma_start(OUT[:], outsb[:])
        fp.release()

        dram.release()
        persist.release()
        psum.release()

    nc.compile()
    return nc


LAST_RESULT = None


def kernel(**inputs):
    global LAST_RESULT
    meta, in_maps, weights = preprocess(inputs)
    nc = build(meta, weights)
    res = run_bass_kernel_spmd(nc, in_maps, list(range(NCORES)))
    LAST_RESULT = res
    outs = []
    for c in range(NCORES):
        o = res.results[c]["out"]
        full = np.zeros((NLOC, 4), np.float32)
        for g in range(NGRP):
            full[g * 128:(g + 1) * 128] = o[:, g, :]
        outs.append(full)
    return np.concatenate(outs, axis=0).astype(np.float32)
